# revision 11
# baseline (speedup 1.0000x reference)
"""LeNet-style CNN (conv5x5+avgpool2+sigmoid x2, then 3 FC layers) on 8 trn2
NeuronCores, pure data parallel over the batch.

v3 key ideas (on top of v2's fused-conv Toeplitz formulation):
- The Activation engine was the measured bottleneck (saturated ~20us sigmoid
  chain; ACT cost = free-size x 0.83ns regardless of dtype).  Half the
  activations now run on the (otherwise idle) DVE as a single custom-DVE op:
  a degree-5 odd minimax polynomial for sigmoid(z)-0.5 (|z|<=1.75 after conv,
  max err 9e-5, far below the fp8 storage noise).  The -0.5 offset is exact:
  it folds into the next layer's bias host-side (b2 += 0.5*sum W2f over the
  DVE-computed kernel rows; Lb1 += 0.5*colsum of the DVE h2 rows of L1).
- DMA issue was serialized on the Sync engine (~850ns per dma_start, 8 DMAs
  before compute could start).  Weights are packed into TWO packed dram
  tensors (fp8 Toeplitz pair; fp32 biases+FC matrices) issued from the Scalar
  engine's HWDGE, while the 4 x chunks issue concurrently from Sync.
- L1 PSUM rotation widened to 3 tiles (q accumulators shrunk to per-half
  [96,512] so everything fits in the 8 psum banks): the PE runs ~3 tiles
  ahead and neither sigmoid engine starves.
- FC tail runs per batch-half so FC1/FC2 ACTs, FC3 matmul+DVE copy, and the
  per-half output DMA all pipeline; engine assignment alternates so the
  p_i -> q_i -> FC critical path never ping-pongs idle engines.
"""

import numpy as np
import ml_dtypes
import concourse.bacc as bacc
import concourse.mybir as mybir
import concourse.tile as tile
from concourse.vector_clock import ScopedClock
from concourse.bass_utils import run_bass_kernel_spmd

F32 = mybir.dt.float32
F32R = mybir.dt.float32r
BF16 = mybir.dt.bfloat16
FP8 = mybir.dt.float8e4
SIG = mybir.ActivationFunctionType.Sigmoid
DR = mybir.MatmulPerfMode.DoubleRow

N_CORES = 8
B_FULL = 8192
NB = B_FULL // N_CORES  # 1024 images per core
HB = 512  # matmul moving-dim tile (one PSUM bank of fp32)
N_WARM = 3

# sigmoid(z) - 0.5 ~= z*(SC1 + u*(SC3 + u*SC5)), u = z^2; minimax |z|<=1.75,
# max abs err 8.9e-5 (z1 in [-1.16,1.33], z2 in [-0.86,0.80] empirically).
SC1 = 0.2496287852838572
SC3 = -0.019776159138807183
SC5 = 0.0012903995739342435

# pi rows whose L1 sigmoid runs on the DVE (stored as sigma-0.5), and qi
# blocks whose L2 sigmoid runs on the DVE.  Odd pi <=> odd kernel row e for
# every qi, so the b2 correction is qi-independent.
USE_DVE = False
DVE_PI = (1, 3, 5, 7, 9, 11) if USE_DVE else ()
DVE_QI = (0, 2) if USE_DVE else ()

# Shrink the semaphore file the NEFF epilogue has to clear: bass kernel sems
# live in [150, MAX_SEM); walrus's GroupResetSemaphores covers [3, max-sem-num)
# one EVENT_SEMAPHORE per sem (~100ns each, split over 5 engines).
MAX_SEM = 176


def _patch_sem_range():
    import concourse.bass as _bass
    from concourse.compiler_utils import get_compiler_flags, set_compiler_flags

    if _bass.get_kernel_semaphore_range().stop != MAX_SEM:
        _orig = _bass.get_kernel_semaphore_range

        def _patched():
            return range(150, MAX_SEM)

        _bass.get_kernel_semaphore_range = _patched
    flags = []
    for f in get_compiler_flags():
        if f.startswith("--internal-backend-options=") and "--max-sem-num" not in f:
            f = f + f" --max-sem-num={MAX_SEM}"
        flags.append(f)
    set_compiler_flags(flags)


_patch_sem_range()


def _register_sig5():
    """Register the SIG5_ANT custom-DVE op (idempotent).  out =
    (in0+in1)*(s0 + u*(s1 + u*imm2)), u=(in0+in1)^2; in1 is the [P,1]
    per-partition bias column."""
    import concourse.dve_ops as dve_ops
    from concourse.dve_spec import Spec, Src0, Src1, C0, C1, C2, sq
    from concourse.dve_spec import lower as spec_lower
    from concourse.dve_uop import DveOpSpec

    if any(op.name == "SIG5_ANT" for op in dve_ops.OPS):
        return next(op for op in dve_ops.OPS if op.name == "SIG5_ANT")

    _z = Src0 + Src1
    _u = sq(_z)
    spec = Spec(
        body=_z * (C0 + _u * (C1 + _u * C2)),
        reference=lambda in0, in1, s0, s1, imm2: (
            (in0.astype(np.float32) + in1)
            * (s0 + (in0.astype(np.float32) + in1) ** 2
               * (s1 + (in0.astype(np.float32) + in1) ** 2 * imm2))
        ),
    )
    row = dve_ops._CUSTOM_DVE_ROW_BASE + len(dve_ops.OPS)
    assert row < 0x20
    shas = {}
    for ver in ("v3", "v4"):
        try:
            compiled = DveOpSpec(
                name="SIG5_ANT",
                opcode=row,
                uops=spec_lower(spec, ver=ver),
                rd1_en=True,
            )
            shas[ver] = compiled.sha(ver)
        except Exception:
            pass
    op = dve_ops.DveOp("SIG5_ANT", spec, subdim=False, uops_sha=shas)
    dve_ops.OPS.append(op)
    dve_ops.CUSTOM_DVE_SPECS["SIG5_ANT"] = spec
    dve_ops._SUB_OPCODE_FOR_NAME["SIG5_ANT"] = row
    return op


SIG5 = _register_sig5()


class SlimTailTileContext(tile.TileContext):
    """Tile's standard teardown emits drain + all-engine barrier + semaphore
    clears + another barrier (~10us on HW). This NEFF executes exactly once
    per load, so the semaphore-reset choreography is dead weight: keep the
    data-completeness drain, do the allocator bookkeeping host-side only."""

    def _drain_and_barrier(self, tick_clock, wait_clock):
        drain_inst = self.nc.sync.drain()
        wait_clock.add_sem_waits(
            drain_inst.ins, ScopedClock({None: tick_clock.global_clock})
        )
        popped = self.nc._tile_sem_poison_stack.pop()
        assert popped is self._sem_poison
        sems = list(self.sems.allocated().values())
        sem_nums = [sm.num for sm in sems]
        self.nc._state.prepend_free_semaphores(sem_nums)
        for poison_set in self.nc._tile_sem_poison_stack:
            poison_set.update(sem_nums)


def _fuse_pool(W):
    """conv(W, stride 1) + 2x2 mean-pool == conv(Wf, stride 2), Wf 6x6."""
    O, C, _, _ = W.shape
    Wf = np.zeros((O, C, 6, 6), np.float32)
    for u in (0, 1):
        for v in (0, 1):
            Wf[:, :, u : u + 5, v : v + 5] += W
    return Wf * 0.25


def _host_weights(W1, b1, W2, b2, L1, Lb1, L2, Lb2, L3, Lb3):
    W1f = _fuse_pool(np.asarray(W1, np.float32))  # [10,1,6,6]
    W2f = _fuse_pool(np.asarray(W2, np.float32))  # [20,10,6,6]

    # Layer 1 Toeplitz: S_k[(m',w), (pj,o)] = W1f[o, 0, 2k+m', w-2pj],
    # merged into 4 zero-padded [120,128] stationaries (partition halves
    # 0-55 / 64-119 are the two kernel-row-pair positions of a 4-row group).
    S = np.zeros((3, 56, 120), np.float32)
    for k in range(3):
        for mp in range(2):
            e = 2 * k + mp
            for pj in range(12):
                for f in range(6):
                    w = 2 * pj + f
                    S[k, mp * 28 + w, pj * 10 : pj * 10 + 10] = W1f[:, 0, e, f]
    t1 = np.zeros((120, 4, 128), np.float32)
    t1[0:56, 0, 0:120] = S[0]
    t1[64:120, 0, 0:120] = S[1]
    t1[0:56, 1, 0:120] = S[2]
    t1[64:120, 2, 0:120] = S[0]
    t1[0:56, 3, 0:120] = S[1]
    t1[64:120, 3, 0:120] = S[2]

    # Layer 2 Toeplitz, fp8 DoubleRow layout: block j2 = 2k+par; free dim
    # padded 80 -> 96 for alignment.
    t2 = np.zeros((120, 6, 96), np.float32)
    for par in range(2):
        for k in range(3):
            e = 2 * k + par
            j2 = 2 * k + par
            for qj in range(4):
                for f in range(6):
                    pj = 2 * qj + f
                    for c in range(10):
                        t2[pj * 10 + c, j2, qj * 20 : qj * 20 + 20] = W2f[:, c, e, f]

    w8 = np.zeros((120, 1088), np.float32)
    w8[:, 0:512] = t1.reshape(120, 512)
    w8[:, 512:1088] = t2.reshape(120, 576)

    b1a = np.asarray(b1, np.float32).reshape(10)
    b2a = np.asarray(b2, np.float32).reshape(20)
    L1a = np.asarray(L1, np.float32)
    L2a = np.asarray(L2, np.float32)

    # h1 rows for pi in DVE_PI hold sigma-0.5: the L2 pre-activation is short
    # by 0.5 * sum of the W2f entries on the DVE kernel rows e (pi = 2qi+e so
    # e odd <=> pi odd, independent of qi).
    b2c = b2a.copy()
    if DVE_PI:
        assert DVE_PI == (1, 3, 5, 7, 9, 11)
        b2c = b2a + 0.5 * W2f[:, :, 1::2, :].sum(axis=(1, 2, 3))

    # h2 blocks qi in DVE_QI hold sigma-0.5: FC1 bias correction is
    # 0.5 * column-sum of the corresponding L1 rows (r = oc*16 + qi*4 + qj).
    rows = [
        oc * 16 + qi * 4 + qj for qi in DVE_QI for qj in range(4) for oc in range(20)
    ]
    lb1c = np.asarray(Lb1, np.float32).reshape(120) + 0.5 * L1a[rows, :].sum(axis=0)

    # fp32 pack [120, 584]:
    # col 0: b1 tiled (120); col 1: b2c (80); col 5: lb1' (120);
    # col 6: lb2 (84); cols 7:17: [L3; Lb3] (85 rows); cols 20:500: FC1
    # permuted (80 rows); cols 500:584: FC2 (120 rows).
    wf = np.zeros((120, 584), np.float32)
    wf[:, 0] = np.tile(b1a, 12)
    wf[0:80, 1] = np.tile(b2c, 4)
    wf[:, 5] = lb1c
    wf[0:84, 6] = np.asarray(Lb2, np.float32).reshape(84)
    wf[0:84, 7:17] = np.asarray(L3, np.float32)
    wf[84, 7:17] = np.asarray(Lb3, np.float32).reshape(10)
    for qi in range(4):
        for qj in range(4):
            for oc in range(20):
                wf[qj * 20 + oc, 20 + qi * 120 : 20 + (qi + 1) * 120] = L1a[
                    oc * 16 + qi * 4 + qj
                ]
    wf[:, 500:584] = L2a  # [120, 84]

    return {
        "w8": np.ascontiguousarray(w8, dtype=ml_dtypes.float8_e4m3),
        "wf": np.ascontiguousarray(wf),
    }


def _build_nc():
    nc = bacc.Bacc()
    xp = nc.dram_tensor("xp", [120, 7, NB], FP8, kind="ExternalInput")
    w8 = nc.dram_tensor("w8", [120, 1088], FP8, kind="ExternalInput")
    wf = nc.dram_tensor("wf", [120, 584], F32R, kind="ExternalInput")
    y = nc.dram_tensor("y", [10, NB], F32, kind="ExternalOutput")

    with SlimTailTileContext(nc) as tc:
        with (
            tc.tile_pool(name="w", bufs=1) as wp,
            tc.tile_pool(name="act", bufs=1) as ap,
            tc.tile_pool(name="ps", bufs=1, space="PSUM") as psp,
        ):
            # --- SBUF residents ---
            warm = wp.tile([128, 640], BF16, tag="warm")
            w8s = wp.tile([120, 1088], FP8, tag="w8")
            wfs = wp.tile([120, 584], F32R, tag="wf")
            xs = ap.tile([120, 7, NB], FP8, tag="xp")
            h1 = ap.tile([120, 12, NB], FP8, tag="h1")
            h2 = ap.tile([80, 4 * NB], F32R, tag="h2")
            h3 = ap.tile([120, NB], F32R, tag="h3")
            h4 = ap.tile([85, NB], F32R, tag="h4")  # row 84 == 1.0 (FC3 bias)
            ys = ap.tile([10, NB], F32, tag="ys")

            t1v = w8s[:, 0:512].rearrange("p (g k) -> p g k", k=128)
            t2v = w8s[:, 512:1088].rearrange("p (g k) -> p g k", k=96)
            b1c = wfs[:, 0:1].bitcast(F32)
            b2col = wfs[0:80, 1:2].bitcast(F32)
            lb1c = wfs[:, 5:6].bitcast(F32)
            lb2c = wfs[0:84, 6:7].bitcast(F32)
            l3s = wfs[0:85, 7:17]
            l1s = wfs[0:80, 20:500]
            l2s = wfs[:, 500:584]

            # --- head: parallel DMA issue (GpSimd SWDGE for weights, Sync
            # for the 4 x chunks; Scalar stays a pure sigmoid chain so the
            # auto-inserted ACT table load runs immediately), PE warm-up ---
            nc.vector.memset(warm[:, :], 0.0)
            nc.gpsimd.dma_start(wfs[:], wf[:])
            nc.gpsimd.dma_start(w8s[:], w8[:])
            nc.sync.dma_start(xs[:, 0:2, :], xp[:, 0:2, :])
            nc.sync.dma_start(xs[:, 2:4, :], xp[:, 2:4, :])
            nc.sync.dma_start(xs[:, 4:6, :], xp[:, 4:6, :])
            nc.sync.dma_start(xs[:, 6:7, :], xp[:, 6:7, :])
            nc.gpsimd.memset(h4[:, :].bitcast(F32), 1.0)
            for _ in range(N_WARM):
                wps = psp.tile([128, HB], F32, tag="l1", bufs=3, name="wps")
                nc.tensor.matmul(
                    wps[:], warm[:, :128], warm[:, 128:640], start=True, stop=True
                )

            # --- layer 1: one DoubleRow K=240 matmul per (pi, half); sigmoid
            # split across ACT (true sigma, even pi) and DVE (sigma-0.5 via
            # SIG5 poly, odd pi). ---
            def l1_mms(pi):
                ps = psp.tile([128, 1024], F32, tag="l1", bufs=3, name=f"psp{pi}")
                g, p = pi // 2, pi % 2
                for h in range(2):
                    b0 = h * HB
                    nc.tensor.matmul(
                        ps[:, b0 : b0 + HB],
                        t1v[:, 2 * p : 2 * p + 2, :],
                        xs[:, g : g + 2, b0 : b0 + HB],
                        start=True,
                        stop=True,
                        perf_mode=DR,
                    )
                return ps

            def l1_act(pi, ps):
                nc.scalar.activation(h1[:, pi, :], ps[0:120, :], SIG, bias=b1c)

            def l1_dve(pi, ps):
                nc.vector._custom_dve(
                    SIG5,
                    out=h1[:, pi, :],
                    in0=ps[0:120, :],
                    in1=b1c,
                    s0=SC1,
                    s1=SC3,
                    imm2=SC5,
                )

            def l1_sig(pi, ps):
                (l1_dve if pi in DVE_PI else l1_act)(pi, ps)

            # --- layer 2: per-half accumulators [96,512] so 3-deep L1
            # rotation + 2-deep q rotation fit the 8 psum banks. ---
            l2_ps = {}

            def l2_mms(qi, h, ks):
                key = (qi, h)
                if key not in l2_ps:
                    l2_ps[key] = psp.tile(
                        [96, HB], F32, tag="q", bufs=2, name=f"psq{qi}_{h}"
                    )
                ps = l2_ps[key]
                b0 = h * HB
                for k in ks:
                    nc.tensor.matmul(
                        ps[:, :],
                        t2v[:, 2 * k : 2 * k + 2, :],
                        h1[:, 2 * (qi + k) : 2 * (qi + k) + 2, b0 : b0 + HB],
                        start=(k == 0),
                        stop=(k == 2),
                        perf_mode=DR,
                    )

            def l2_sig(qi, h):
                ps = l2_ps[(qi, h)]
                dst = h2[:, qi * NB + h * HB : qi * NB + h * HB + HB]
                if qi in DVE_QI:
                    nc.vector._custom_dve(
                        SIG5,
                        out=dst,
                        in0=ps[0:80, :],
                        in1=b2col,
                        s0=SC1,
                        s1=SC3,
                        imm2=SC5,
                    )
                else:
                    nc.scalar.activation(dst, ps[0:80, :], SIG, bias=b2col)

            # --- emission schedule: PE runs p-tiles as x chunks land, with
            # q accumulations slotted between; all 12 L1 sigmoids are emitted
            # before any q sigmoid on both engines so the late p tiles (which
            # gate q2/q3 and thus the FC tail) are never priority-inverted.
            for pi in (0, 1, 2, 3, 4, 5):
                l1_sig(pi, l1_mms(pi))
            for h in range(2):
                l2_mms(0, h, (0, 1))
            for pi in (6, 7):
                l1_sig(pi, l1_mms(pi))
            for h in range(2):
                l2_mms(0, h, (2,))
                l2_mms(1, h, (0, 1))
            for pi in (8, 9):
                l1_sig(pi, l1_mms(pi))
            for h in range(2):
                l2_mms(1, h, (2,))
                l2_mms(2, h, (0, 1))
            for pi in (10, 11):
                l1_sig(pi, l1_mms(pi))
            for h in range(2):
                l2_mms(2, h, (2,))
                l2_mms(3, h, (0, 1, 2))
            for qi in range(4):
                for h in range(2):
                    l2_sig(qi, h)

            # --- FC tail, per batch-half: FC1+FC2 ACT sigmoids, FC3 matmul
            # + DVE copy + per-half output DMA. ---
            ps1 = [
                psp.tile([120, HB], F32, tag="l1", bufs=3, name=f"ps1{h}")
                for h in range(2)
            ]
            ps2 = [
                psp.tile([84, HB], F32, tag="q", bufs=2, name=f"ps2{h}")
                for h in range(2)
            ]
            ps3 = [
                psp.tile([10, HB], F32, tag="l1", bufs=3, name=f"ps3{h}")
                for h in range(2)
            ]
            for h in range(2):
                b0 = h * HB
                for qi in range(4):
                    nc.tensor.matmul(
                        ps1[h][:, :],
                        l1s[:, qi * 120 : (qi + 1) * 120],
                        h2[:, qi * NB + b0 : qi * NB + b0 + HB],
                        start=(qi == 0),
                        stop=(qi == 3),
                    )
                nc.scalar.activation(h3[:, b0 : b0 + HB], ps1[h][:, :], SIG, bias=lb1c)
            for h in range(2):
                b0 = h * HB
                nc.tensor.matmul(
                    ps2[h][:, :], l2s, h3[:, b0 : b0 + HB], start=True, stop=True
                )
                nc.scalar.activation(
                    h4[0:84, b0 : b0 + HB], ps2[h][:, :], SIG, bias=lb2c
                )
            for h in range(2):
                b0 = h * HB
                nc.tensor.matmul(
                    ps3[h][:, :], l3s, h4[:, b0 : b0 + HB], start=True, stop=True
                )
                nc.vector.tensor_copy(ys[:, b0 : b0 + HB], ps3[h][:, :])
                nc.sync.dma_start(y[:, b0 : b0 + HB], ys[:, b0 : b0 + HB])
    nc.compile()
    return nc


_NC_CACHE = None


def _get_nc():
    global _NC_CACHE
    if _NC_CACHE is None:
        _NC_CACHE = _build_nc()
    return _NC_CACHE


def _make_in_maps(x, W1, b1, W2, b2, L1, Lb1, L2, Lb2, L3, Lb3):
    wmap = _host_weights(W1, b1, W2, b2, L1, Lb1, L2, Lb2, L3, Lb3)
    x = np.asarray(x, dtype=np.float32)
    in_maps = []
    for c in range(N_CORES):
        xc = x[c * NB : (c + 1) * NB, 0]  # [NB, 28, 28]
        # rows r = 4g + m; partitions: m in {0,1} -> 0:56, m in {2,3} -> 64:120
        v = xc.reshape(NB, 7, 4, 28).transpose(2, 3, 1, 0).reshape(112, 7, NB)
        xpc = np.zeros((120, 7, NB), dtype=ml_dtypes.float8_e4m3)
        xpc[0:56] = v[0:56]
        xpc[64:120] = v[56:112]
        m = {"xp": xpc}
        m.update(wmap)
        in_maps.append(m)
    return in_maps


def _run(trace=False, **inputs):
    global _NC_CACHE
    nc = _get_nc()
    in_maps = _make_in_maps(**inputs)
    res = run_bass_kernel_spmd(nc, in_maps, list(range(N_CORES)), trace=trace)
    # the slim teardown leaves semaphores dirty; force a fresh NEFF if
    # kernel() is ever called again in this process
    _NC_CACHE = None
    outs = []
    for i in range(N_CORES):
        yc = res.results[i]["y"]  # [10, NB]
        outs.append(yc.T)
    out = np.ascontiguousarray(np.concatenate(outs, axis=0))
    return out, res


def kernel(**inputs):
    out, _ = _run(trace=False, **inputs)
    return out


# revision 18
# speedup vs baseline: 1.0466x; 1.0466x over previous
"""LeNet-style CNN (conv5x5+avgpool2+sigmoid x2, then 3 FC layers) on 8 trn2
NeuronCores, pure data parallel over the batch.

v3 key ideas (on top of v2's fused-conv Toeplitz formulation):
- The Activation engine was the measured bottleneck (saturated ~20us sigmoid
  chain; ACT cost = free-size x 0.83ns regardless of dtype).  Half the
  activations now run on the (otherwise idle) DVE as a single custom-DVE op:
  a degree-5 odd minimax polynomial for sigmoid(z)-0.5 (|z|<=1.75 after conv,
  max err 9e-5, far below the fp8 storage noise).  The -0.5 offset is exact:
  it folds into the next layer's bias host-side (b2 += 0.5*sum W2f over the
  DVE-computed kernel rows; Lb1 += 0.5*colsum of the DVE h2 rows of L1).
- DMA issue was serialized on the Sync engine (~850ns per dma_start, 8 DMAs
  before compute could start).  Weights are packed into TWO packed dram
  tensors (fp8 Toeplitz pair; fp32 biases+FC matrices) issued from the Scalar
  engine's HWDGE, while the 4 x chunks issue concurrently from Sync.
- L1 PSUM rotation widened to 3 tiles (q accumulators shrunk to per-half
  [96,512] so everything fits in the 8 psum banks): the PE runs ~3 tiles
  ahead and neither sigmoid engine starves.
- FC tail runs per batch-half so FC1/FC2 ACTs, FC3 matmul+DVE copy, and the
  per-half output DMA all pipeline; engine assignment alternates so the
  p_i -> q_i -> FC critical path never ping-pongs idle engines.
"""

import numpy as np
import ml_dtypes
import concourse.bacc as bacc
import concourse.mybir as mybir
import concourse.tile as tile
from concourse.vector_clock import ScopedClock
from concourse.bass_utils import run_bass_kernel_spmd

F32 = mybir.dt.float32
F32R = mybir.dt.float32r
BF16 = mybir.dt.bfloat16
FP8 = mybir.dt.float8e4
SIG = mybir.ActivationFunctionType.Sigmoid
DR = mybir.MatmulPerfMode.DoubleRow

N_CORES = 8
B_FULL = 8192
NB = B_FULL // N_CORES  # 1024 images per core
HB = 512  # matmul moving-dim tile (one PSUM bank of fp32)
N_WARM = 4

# sigmoid(z) - 0.5 ~= z*(SC1 + u*(SC3 + u*SC5)), u = z^2; minimax |z|<=1.75,
# max abs err 8.9e-5 (z1 in [-1.16,1.33], z2 in [-0.86,0.80] empirically).
SC1 = 0.2496287852838572
SC3 = -0.019776159138807183
SC5 = 0.0012903995739342435

# pi rows whose L1 sigmoid runs on the DVE (stored as sigma-0.5), and qi
# blocks whose L2 sigmoid runs on the DVE.  Odd pi <=> odd kernel row e for
# every qi, so the b2 correction is qi-independent.
USE_DVE = True
DVE_PI = (1, 3, 5, 7, 9, 11) if USE_DVE else ()
DVE_QI = (0, 2) if USE_DVE else ()


def _register_sig5():
    """Register the SIG5_ANT custom-DVE op (idempotent).  out =
    z*(s0 + u*(s1 + u*imm2)), z = in0 + in1, u = z^2; in1 is the [P,1]
    per-partition bias column, routed via the C3 -> Latch(Src1) spill
    (read once per partition through the swap flop — a bare streaming
    Src1 with a length-1 in1 underruns and hangs the engine)."""
    import concourse.dve_ops as dve_ops
    from concourse.dve_spec import Spec, Src0, C0, C1, C2, C3, sq
    from concourse.dve_spec import lower as spec_lower
    from concourse.dve_spec import _spill_c3_to_src1
    from concourse.dve_uop import DveOpSpec

    if any(op.name == "SIG5_ANT" for op in dve_ops.OPS):
        return next(op for op in dve_ops.OPS if op.name == "SIG5_ANT")

    _z = Src0 + C3
    _u = sq(_z)
    spec = Spec(
        body=_spill_c3_to_src1(_z * (C0 + _u * (C1 + _u * C2))),
        reference=lambda in0, in1, s0, s1, imm2: (
            (in0.astype(np.float32) + in1)
            * (s0 + (in0.astype(np.float32) + in1) ** 2
               * (s1 + (in0.astype(np.float32) + in1) ** 2 * imm2))
        ),
    )
    row = dve_ops._CUSTOM_DVE_ROW_BASE + len(dve_ops.OPS)
    assert row < 0x20
    shas = {}
    for ver in ("v3", "v4"):
        try:
            compiled = DveOpSpec(
                name="SIG5_ANT",
                opcode=row,
                uops=spec_lower(spec, ver=ver),
                rd1_en=True,
            )
            shas[ver] = compiled.sha(ver)
        except Exception:
            pass
    op = dve_ops.DveOp("SIG5_ANT", spec, subdim=False, uops_sha=shas)
    dve_ops.OPS.append(op)
    dve_ops.CUSTOM_DVE_SPECS["SIG5_ANT"] = spec
    dve_ops._SUB_OPCODE_FOR_NAME["SIG5_ANT"] = row
    return op


SIG5 = _register_sig5()


class SlimTailTileContext(tile.TileContext):
    """Tile's standard teardown emits drain + all-engine barrier + semaphore
    clears + another barrier (~10us on HW). This NEFF executes exactly once
    per load, so the semaphore-reset choreography is dead weight: keep the
    data-completeness drain, do the allocator bookkeeping host-side only."""

    def _drain_and_barrier(self, tick_clock, wait_clock):
        drain_inst = self.nc.sync.drain()
        wait_clock.add_sem_waits(
            drain_inst.ins, ScopedClock({None: tick_clock.global_clock})
        )
        popped = self.nc._tile_sem_poison_stack.pop()
        assert popped is self._sem_poison
        sems = list(self.sems.allocated().values())
        sem_nums = [sm.num for sm in sems]
        self.nc._state.prepend_free_semaphores(sem_nums)
        for poison_set in self.nc._tile_sem_poison_stack:
            poison_set.update(sem_nums)


def _fuse_pool(W):
    """conv(W, stride 1) + 2x2 mean-pool == conv(Wf, stride 2), Wf 6x6."""
    O, C, _, _ = W.shape
    Wf = np.zeros((O, C, 6, 6), np.float32)
    for u in (0, 1):
        for v in (0, 1):
            Wf[:, :, u : u + 5, v : v + 5] += W
    return Wf * 0.25


def _host_weights(W1, b1, W2, b2, L1, Lb1, L2, Lb2, L3, Lb3):
    W1f = _fuse_pool(np.asarray(W1, np.float32))  # [10,1,6,6]
    W2f = _fuse_pool(np.asarray(W2, np.float32))  # [20,10,6,6]

    # Layer 1 Toeplitz: S_k[(m',w), (pj,o)] = W1f[o, 0, 2k+m', w-2pj],
    # merged into 4 zero-padded [120,128] stationaries (partition halves
    # 0-55 / 64-119 are the two kernel-row-pair positions of a 4-row group).
    S = np.zeros((3, 56, 120), np.float32)
    for k in range(3):
        for mp in range(2):
            e = 2 * k + mp
            for pj in range(12):
                for f in range(6):
                    w = 2 * pj + f
                    S[k, mp * 28 + w, pj * 10 : pj * 10 + 10] = W1f[:, 0, e, f]
    t1 = np.zeros((120, 4, 128), np.float32)
    t1[0:56, 0, 0:120] = S[0]
    t1[64:120, 0, 0:120] = S[1]
    t1[0:56, 1, 0:120] = S[2]
    t1[64:120, 2, 0:120] = S[0]
    t1[0:56, 3, 0:120] = S[1]
    t1[64:120, 3, 0:120] = S[2]

    # Layer 2 Toeplitz, fp8 DoubleRow layout: block j2 = 2k+par; free dim
    # padded 80 -> 96 for alignment.
    t2 = np.zeros((120, 6, 96), np.float32)
    for par in range(2):
        for k in range(3):
            e = 2 * k + par
            j2 = 2 * k + par
            for qj in range(4):
                for f in range(6):
                    pj = 2 * qj + f
                    for c in range(10):
                        t2[pj * 10 + c, j2, qj * 20 : qj * 20 + 20] = W2f[:, c, e, f]

    w8 = np.zeros((120, 1088), np.float32)
    w8[:, 0:512] = t1.reshape(120, 512)
    w8[:, 512:1088] = t2.reshape(120, 576)

    b1a = np.asarray(b1, np.float32).reshape(10)
    b2a = np.asarray(b2, np.float32).reshape(20)
    L1a = np.asarray(L1, np.float32)
    L2a = np.asarray(L2, np.float32)

    # h1 rows for pi in DVE_PI hold sigma-0.5: the L2 pre-activation is short
    # by 0.5 * sum of the W2f entries on the DVE kernel rows e (pi = 2qi+e so
    # e odd <=> pi odd, independent of qi).
    b2c = b2a.copy()
    if DVE_PI:
        assert DVE_PI == (1, 3, 5, 7, 9, 11)
        b2c = b2a + 0.5 * W2f[:, :, 1::2, :].sum(axis=(1, 2, 3))

    # h2 blocks qi in DVE_QI hold sigma-0.5: FC1 bias correction is
    # 0.5 * column-sum of the corresponding L1 rows (r = oc*16 + qi*4 + qj).
    rows = [
        oc * 16 + qi * 4 + qj for qi in DVE_QI for qj in range(4) for oc in range(20)
    ]
    lb1c = np.asarray(Lb1, np.float32).reshape(120) + 0.5 * L1a[rows, :].sum(axis=0)

    # fp32 pack [120, 584]:
    # col 0: b1 tiled (120); col 1: b2c (80); col 5: lb1' (120);
    # col 6: lb2 (84); cols 7:17: [L3; Lb3] (85 rows); cols 20:500: FC1
    # permuted (80 rows); cols 500:584: FC2 (120 rows).
    wf = np.zeros((120, 584), np.float32)
    wf[:, 0] = np.tile(b1a, 12)
    wf[0:80, 1] = np.tile(b2c, 4)
    wf[:, 5] = lb1c
    wf[0:84, 6] = np.asarray(Lb2, np.float32).reshape(84)
    wf[0:84, 7:17] = np.asarray(L3, np.float32)
    wf[84, 7:17] = np.asarray(Lb3, np.float32).reshape(10)
    for qi in range(4):
        for qj in range(4):
            for oc in range(20):
                wf[qj * 20 + oc, 20 + qi * 120 : 20 + (qi + 1) * 120] = L1a[
                    oc * 16 + qi * 4 + qj
                ]
    wf[:, 500:584] = L2a  # [120, 84]

    return {
        "w8": np.ascontiguousarray(w8, dtype=ml_dtypes.float8_e4m3),
        "wf": np.ascontiguousarray(wf),
    }


def _build_nc():
    nc = bacc.Bacc()
    xp = nc.dram_tensor("xp", [120, 7, NB], FP8, kind="ExternalInput")
    w8 = nc.dram_tensor("w8", [120, 1088], FP8, kind="ExternalInput")
    wf = nc.dram_tensor("wf", [120, 584], F32R, kind="ExternalInput")
    y = nc.dram_tensor("y", [10, NB], F32, kind="ExternalOutput")

    with SlimTailTileContext(nc) as tc:
        with (
            tc.tile_pool(name="w", bufs=1) as wp,
            tc.tile_pool(name="act", bufs=1) as ap,
            tc.tile_pool(name="ps", bufs=1, space="PSUM") as psp,
        ):
            # --- SBUF residents ---
            warm = wp.tile([128, 640], BF16, tag="warm")
            w8s = wp.tile([120, 1088], FP8, tag="w8")
            wfs = wp.tile([120, 584], F32R, tag="wf")
            xs = ap.tile([120, 7, NB], FP8, tag="xp")
            h1 = ap.tile([120, 12, NB], FP8, tag="h1")
            h2 = ap.tile([80, 4 * NB], F32R, tag="h2")
            h3 = ap.tile([120, NB], F32R, tag="h3")
            h4 = ap.tile([85, NB], F32R, tag="h4")  # row 84 == 1.0 (FC3 bias)
            ys = ap.tile([10, NB], F32, tag="ys")

            t1v = w8s[:, 0:512].rearrange("p (g k) -> p g k", k=128)
            t2v = w8s[:, 512:1088].rearrange("p (g k) -> p g k", k=96)
            b1c = wfs[:, 0:1].bitcast(F32)
            b2col = wfs[0:80, 1:2].bitcast(F32)
            lb1c = wfs[:, 5:6].bitcast(F32)
            lb2c = wfs[0:84, 6:7].bitcast(F32)
            l3s = wfs[0:85, 7:17]
            l1s = wfs[0:80, 20:500]
            l2s = wfs[:, 500:584]

            # --- head: parallel DMA issue across three engines.  Scalar
            # HWDGE takes the two tensors that gate the first L1 matmul
            # (x01, w8) so compute starts earliest; its auto-inserted ACT
            # table load then runs before the first sigmoid needs it.  Sync
            # HWDGE takes wf + the mid x chunks; the last x chunk rides the
            # slow-but-parallel GpSimd SWDGE.  PE warm-up covers the HAM
            # clock ramp until data lands. ---
            nc.gpsimd.memset(warm[:, :], 0.0)
            nc.scalar.dma_start(xs[:, 0:2, :], xp[:, 0:2, :])
            nc.scalar.dma_start(w8s[:], w8[:])
            nc.sync.dma_start(wfs[:], wf[:])
            nc.sync.dma_start(xs[:, 2:4, :], xp[:, 2:4, :])
            nc.sync.dma_start(xs[:, 4:6, :], xp[:, 4:6, :])
            nc.gpsimd.dma_start(xs[:, 6:7, :], xp[:, 6:7, :])
            nc.gpsimd.memset(h4[:, :].bitcast(F32), 1.0)
            for _ in range(N_WARM):
                wps = psp.tile([128, HB], F32, tag="l1", bufs=3, name="wps")
                nc.tensor.matmul(
                    wps[:], warm[:, :128], warm[:, 128:640], start=True, stop=True
                )

            # --- layer 1: one DoubleRow K=240 matmul per (pi, half); sigmoid
            # split across ACT (true sigma, even pi) and DVE (sigma-0.5 via
            # SIG5 poly, odd pi). ---
            def l1_mms(pi):
                ps = psp.tile([128, 1024], F32, tag="l1", bufs=3, name=f"psp{pi}")
                g, p = pi // 2, pi % 2
                for h in range(2):
                    b0 = h * HB
                    nc.tensor.matmul(
                        ps[:, b0 : b0 + HB],
                        t1v[:, 2 * p : 2 * p + 2, :],
                        xs[:, g : g + 2, b0 : b0 + HB],
                        start=True,
                        stop=True,
                        perf_mode=DR,
                    )
                return ps

            def l1_act(pi, ps):
                nc.scalar.activation(h1[:, pi, :], ps[0:120, :], SIG, bias=b1c)

            def l1_dve(pi, ps):
                nc.vector._custom_dve(
                    SIG5,
                    out=h1[:, pi, :],
                    in0=ps[0:120, :],
                    in1=b1c,
                    s0=SC1,
                    s1=SC3,
                    imm2=SC5,
                )

            def l1_sig(pi, ps):
                (l1_dve if pi in DVE_PI else l1_act)(pi, ps)

            # --- layer 2: per-half accumulators [96,512] so 3-deep L1
            # rotation + 2-deep q rotation fit the 8 psum banks. ---
            l2_ps = {}

            def l2_mms(qi, h, ks):
                key = (qi, h)
                if key not in l2_ps:
                    l2_ps[key] = psp.tile(
                        [96, HB], F32, tag="q", bufs=2, name=f"psq{qi}_{h}"
                    )
                ps = l2_ps[key]
                b0 = h * HB
                for k in ks:
                    nc.tensor.matmul(
                        ps[:, :],
                        t2v[:, 2 * k : 2 * k + 2, :],
                        h1[:, 2 * (qi + k) : 2 * (qi + k) + 2, b0 : b0 + HB],
                        start=(k == 0),
                        stop=(k == 2),
                        perf_mode=DR,
                    )

            def l2_sig(qi, h):
                ps = l2_ps[(qi, h)]
                dst = h2[:, qi * NB + h * HB : qi * NB + h * HB + HB]
                if qi in DVE_QI:
                    nc.vector._custom_dve(
                        SIG5,
                        out=dst,
                        in0=ps[0:80, :],
                        in1=b2col,
                        s0=SC1,
                        s1=SC3,
                        imm2=SC5,
                    )
                else:
                    nc.scalar.activation(dst, ps[0:80, :], SIG, bias=b2col)

            # --- emission schedule: PE runs p-tiles as x chunks land, with
            # q accumulations slotted between; all 12 L1 sigmoids are emitted
            # before any q sigmoid on both engines so the late p tiles (which
            # gate q2/q3 and thus the FC tail) are never priority-inverted.
            for pi in (0, 1, 2, 3, 4, 5):
                l1_sig(pi, l1_mms(pi))
            for h in range(2):
                l2_mms(0, h, (0, 1))
            for pi in (6, 7):
                l1_sig(pi, l1_mms(pi))
            for h in range(2):
                l2_mms(0, h, (2,))
                l2_mms(1, h, (0, 1))
            for pi in (8, 9):
                l1_sig(pi, l1_mms(pi))
            for h in range(2):
                l2_mms(1, h, (2,))
                l2_mms(2, h, (0, 1))
            for pi in (10, 11):
                l1_sig(pi, l1_mms(pi))
            for h in range(2):
                l2_mms(2, h, (2,))
                l2_mms(3, h, (0, 1, 2))
            for qi in range(4):
                for h in range(2):
                    l2_sig(qi, h)

            # --- FC tail, per batch-half: FC1+FC2 ACT sigmoids, FC3 matmul
            # + DVE copy + per-half output DMA. ---
            ps1 = [
                psp.tile([120, HB], F32, tag="l1", bufs=3, name=f"ps1{h}")
                for h in range(2)
            ]
            ps2 = [
                psp.tile([84, HB], F32, tag="q", bufs=2, name=f"ps2{h}")
                for h in range(2)
            ]
            ps3 = [
                psp.tile([10, HB], F32, tag="l1", bufs=3, name=f"ps3{h}")
                for h in range(2)
            ]
            for h in range(2):
                b0 = h * HB
                for qi in range(4):
                    nc.tensor.matmul(
                        ps1[h][:, :],
                        l1s[:, qi * 120 : (qi + 1) * 120],
                        h2[:, qi * NB + b0 : qi * NB + b0 + HB],
                        start=(qi == 0),
                        stop=(qi == 3),
                    )
                nc.scalar.activation(h3[:, b0 : b0 + HB], ps1[h][:, :], SIG, bias=lb1c)
            for h in range(2):
                b0 = h * HB
                nc.tensor.matmul(
                    ps2[h][:, :], l2s, h3[:, b0 : b0 + HB], start=True, stop=True
                )
                nc.scalar.activation(
                    h4[0:84, b0 : b0 + HB], ps2[h][:, :], SIG, bias=lb2c
                )
            for h in range(2):
                b0 = h * HB
                nc.tensor.matmul(
                    ps3[h][:, :], l3s, h4[:, b0 : b0 + HB], start=True, stop=True
                )
                nc.vector.tensor_copy(ys[:, b0 : b0 + HB], ps3[h][:, :])
                nc.sync.dma_start(y[:, b0 : b0 + HB], ys[:, b0 : b0 + HB])
    nc.compile()
    return nc


_NC_CACHE = None


def _get_nc():
    global _NC_CACHE
    if _NC_CACHE is None:
        _NC_CACHE = _build_nc()
    return _NC_CACHE


def _make_in_maps(x, W1, b1, W2, b2, L1, Lb1, L2, Lb2, L3, Lb3):
    wmap = _host_weights(W1, b1, W2, b2, L1, Lb1, L2, Lb2, L3, Lb3)
    x = np.asarray(x, dtype=np.float32)
    in_maps = []
    for c in range(N_CORES):
        xc = x[c * NB : (c + 1) * NB, 0]  # [NB, 28, 28]
        # rows r = 4g + m; partitions: m in {0,1} -> 0:56, m in {2,3} -> 64:120
        v = xc.reshape(NB, 7, 4, 28).transpose(2, 3, 1, 0).reshape(112, 7, NB)
        xpc = np.zeros((120, 7, NB), dtype=ml_dtypes.float8_e4m3)
        xpc[0:56] = v[0:56]
        xpc[64:120] = v[56:112]
        m = {"xp": xpc}
        m.update(wmap)
        in_maps.append(m)
    return in_maps


def _run(trace=False, **inputs):
    global _NC_CACHE
    nc = _get_nc()
    in_maps = _make_in_maps(**inputs)
    res = run_bass_kernel_spmd(nc, in_maps, list(range(N_CORES)), trace=trace)
    # the slim teardown leaves semaphores dirty; force a fresh NEFF if
    # kernel() is ever called again in this process
    _NC_CACHE = None
    outs = []
    for i in range(N_CORES):
        yc = res.results[i]["y"]  # [10, NB]
        outs.append(yc.T)
    out = np.ascontiguousarray(np.concatenate(outs, axis=0))
    return out, res


def kernel(**inputs):
    out, _ = _run(trace=False, **inputs)
    return out


# revision 19
# speedup vs baseline: 1.1890x; 1.1361x over previous
"""LeNet-style CNN (conv5x5+avgpool2+sigmoid x2, then 3 FC layers) on 8 trn2
NeuronCores, pure data parallel over the batch.

v5 key ideas (on top of v2's fused-conv Toeplitz formulation):
- The Activation engine was the measured bottleneck (saturated ~20us sigmoid
  chain; ACT cost = free-size x 0.83ns regardless of dtype).  Half the
  activations now run on the (otherwise idle) DVE as a single custom-DVE op:
  a degree-5 odd minimax polynomial for sigmoid(z)-0.5 (|z|<=1.75 after conv,
  max err 9e-5, far below the fp8 storage noise).  The -0.5 offset is exact:
  it folds into the next layer's bias host-side (b2 += 0.5*sum W2f over the
  DVE-computed kernel rows; Lb1 += 0.5*colsum of the DVE h2 rows of L1).
  The per-partition bias rides in1 via the C3 -> Latch(Src1) spill.
- Everything is computed per batch-half ([*, 512] PSUM tiles): the L1 ring
  (4 bufs) maps even pi to ACT and odd pi to DVE on fixed slots, so the
  write-after-read chain never crosses engines and neither sigmoid engine
  ever waits on the other; same for the 4-slot q ring.  Halving also lets
  FC1's h0 chain start after the q*h0 sigmoids only.
- DMA order is strict first-use order on the fast Sync HWDGE ring (biases,
  conv weights, then x in 5 slices); the late-needed FC matrices ride the
  slow-but-parallel GpSimd SWDGE; the Scalar engine stays a pure sigmoid
  chain so its two auto-inserted ACT table loads finish before data lands.
- 8 short warm-up matmuls bridge the PE from the preamble to first data so
  the HAM clock ramp is never interrupted (an idle gap demotes the PE to
  1.2 GHz for several microseconds).
"""

import numpy as np
import ml_dtypes
import concourse.bacc as bacc
import concourse.mybir as mybir
import concourse.tile as tile
from concourse.vector_clock import ScopedClock
from concourse.bass_utils import run_bass_kernel_spmd

F32 = mybir.dt.float32
F32R = mybir.dt.float32r
BF16 = mybir.dt.bfloat16
FP8 = mybir.dt.float8e4
SIG = mybir.ActivationFunctionType.Sigmoid
DR = mybir.MatmulPerfMode.DoubleRow

N_CORES = 8
B_FULL = 8192
NB = B_FULL // N_CORES  # 1024 images per core
HB = 512  # batch-half: the PSUM tile moving size
N_WARM = 8
WARM_COLS = 256

# sigmoid(z) - 0.5 ~= z*(SC1 + u*(SC3 + u*SC5)), u = z^2; minimax |z|<=1.75,
# max abs err 8.9e-5 (z1 in [-1.16,1.33], z2 in [-0.86,0.80] empirically).
SC1 = 0.2496287852838572
SC3 = -0.019776159138807183
SC5 = 0.0012903995739342435

# pi rows whose L1 sigmoid runs on the DVE (stored as sigma-0.5), and qi
# blocks whose L2 sigmoid runs on the DVE.  Odd pi <=> odd kernel row e for
# every qi, so the b2 correction is qi-independent.
USE_DVE = True
DVE_PI = (1, 3, 5, 7, 9, 11) if USE_DVE else ()
DVE_QI = (0, 2) if USE_DVE else ()


def _register_sig5():
    """Register the SIG5_ANT custom-DVE op (idempotent).  out =
    z*(s0 + u*(s1 + u*imm2)), z = in0 + in1, u = z^2; in1 is the [P,1]
    per-partition bias column, routed via the C3 -> Latch(Src1) spill
    (read once per partition through the swap flop — a bare streaming
    Src1 with a length-1 in1 underruns and hangs the engine)."""
    import concourse.dve_ops as dve_ops
    from concourse.dve_spec import Spec, Src0, C0, C1, C2, C3, sq
    from concourse.dve_spec import lower as spec_lower
    from concourse.dve_spec import _spill_c3_to_src1
    from concourse.dve_uop import DveOpSpec

    if any(op.name == "SIG5_ANT" for op in dve_ops.OPS):
        return next(op for op in dve_ops.OPS if op.name == "SIG5_ANT")

    _z = Src0 + C3
    _u = sq(_z)
    spec = Spec(
        body=_spill_c3_to_src1(_z * (C0 + _u * (C1 + _u * C2))),
        reference=lambda in0, in1, s0, s1, imm2: (
            (in0.astype(np.float32) + in1)
            * (s0 + (in0.astype(np.float32) + in1) ** 2
               * (s1 + (in0.astype(np.float32) + in1) ** 2 * imm2))
        ),
    )
    row = dve_ops._CUSTOM_DVE_ROW_BASE + len(dve_ops.OPS)
    assert row < 0x20
    shas = {}
    for ver in ("v3", "v4"):
        try:
            compiled = DveOpSpec(
                name="SIG5_ANT",
                opcode=row,
                uops=spec_lower(spec, ver=ver),
                rd1_en=True,
            )
            shas[ver] = compiled.sha(ver)
        except Exception:
            pass
    op = dve_ops.DveOp("SIG5_ANT", spec, subdim=False, uops_sha=shas)
    dve_ops.OPS.append(op)
    dve_ops.CUSTOM_DVE_SPECS["SIG5_ANT"] = spec
    dve_ops._SUB_OPCODE_FOR_NAME["SIG5_ANT"] = row
    return op


SIG5 = _register_sig5()


class SlimTailTileContext(tile.TileContext):
    """Tile's standard teardown emits drain + all-engine barrier + semaphore
    clears + another barrier (~10us on HW). This NEFF executes exactly once
    per load, so the semaphore-reset choreography is dead weight: keep the
    data-completeness drain, do the allocator bookkeeping host-side only."""

    def _drain_and_barrier(self, tick_clock, wait_clock):
        drain_inst = self.nc.sync.drain()
        wait_clock.add_sem_waits(
            drain_inst.ins, ScopedClock({None: tick_clock.global_clock})
        )
        popped = self.nc._tile_sem_poison_stack.pop()
        assert popped is self._sem_poison
        sems = list(self.sems.allocated().values())
        sem_nums = [sm.num for sm in sems]
        self.nc._state.prepend_free_semaphores(sem_nums)
        for poison_set in self.nc._tile_sem_poison_stack:
            poison_set.update(sem_nums)


def _fuse_pool(W):
    """conv(W, stride 1) + 2x2 mean-pool == conv(Wf, stride 2), Wf 6x6."""
    O, C, _, _ = W.shape
    Wf = np.zeros((O, C, 6, 6), np.float32)
    for u in (0, 1):
        for v in (0, 1):
            Wf[:, :, u : u + 5, v : v + 5] += W
    return Wf * 0.25


def _host_weights(W1, b1, W2, b2, L1, Lb1, L2, Lb2, L3, Lb3):
    W1f = _fuse_pool(np.asarray(W1, np.float32))  # [10,1,6,6]
    W2f = _fuse_pool(np.asarray(W2, np.float32))  # [20,10,6,6]

    # Layer 1 Toeplitz: S_k[(m',w), (pj,o)] = W1f[o, 0, 2k+m', w-2pj],
    # merged into 4 zero-padded [120,128] stationaries (partition halves
    # 0-55 / 64-119 are the two kernel-row-pair positions of a 4-row group).
    S = np.zeros((3, 56, 120), np.float32)
    for k in range(3):
        for mp in range(2):
            e = 2 * k + mp
            for pj in range(12):
                for f in range(6):
                    w = 2 * pj + f
                    S[k, mp * 28 + w, pj * 10 : pj * 10 + 10] = W1f[:, 0, e, f]
    t1 = np.zeros((120, 4, 128), np.float32)
    t1[0:56, 0, 0:120] = S[0]
    t1[64:120, 0, 0:120] = S[1]
    t1[0:56, 1, 0:120] = S[2]
    t1[64:120, 2, 0:120] = S[0]
    t1[0:56, 3, 0:120] = S[1]
    t1[64:120, 3, 0:120] = S[2]

    # Layer 2 Toeplitz, fp8 DoubleRow layout: block j2 = 2k+par; free dim
    # padded 80 -> 96 for alignment.
    t2 = np.zeros((120, 6, 96), np.float32)
    for par in range(2):
        for k in range(3):
            e = 2 * k + par
            j2 = 2 * k + par
            for qj in range(4):
                for f in range(6):
                    pj = 2 * qj + f
                    for c in range(10):
                        t2[pj * 10 + c, j2, qj * 20 : qj * 20 + 20] = W2f[:, c, e, f]

    w8 = np.zeros((120, 1088), np.float32)
    w8[:, 0:512] = t1.reshape(120, 512)
    w8[:, 512:1088] = t2.reshape(120, 576)

    b1a = np.asarray(b1, np.float32).reshape(10)
    b2a = np.asarray(b2, np.float32).reshape(20)
    L1a = np.asarray(L1, np.float32)
    L2a = np.asarray(L2, np.float32)

    # h1 rows for pi in DVE_PI hold sigma-0.5: the L2 pre-activation is short
    # by 0.5 * sum of the W2f entries on the DVE kernel rows e (pi = 2qi+e so
    # e odd <=> pi odd, independent of qi).
    b2c = b2a.copy()
    if DVE_PI:
        assert DVE_PI == (1, 3, 5, 7, 9, 11)
        b2c = b2a + 0.5 * W2f[:, :, 1::2, :].sum(axis=(1, 2, 3))

    # h2 blocks qi in DVE_QI hold sigma-0.5: FC1 bias correction is
    # 0.5 * column-sum of the corresponding L1 rows (r = oc*16 + qi*4 + qj).
    rows = [
        oc * 16 + qi * 4 + qj for qi in DVE_QI for qj in range(4) for oc in range(20)
    ]
    lb1c = np.asarray(Lb1, np.float32).reshape(120) + (
        0.5 * L1a[rows, :].sum(axis=0) if rows else 0.0
    )

    # bias pack [120, 20] (fp32): col 0: b1 tiled (120); col 1: b2c (80);
    # col 5: lb1' (120); col 6: lb2 (84); cols 7:17: [L3; Lb3] (85 rows).
    wb = np.zeros((120, 20), np.float32)
    wb[:, 0] = np.tile(b1a, 12)
    wb[0:80, 1] = np.tile(b2c, 4)
    wb[:, 5] = lb1c
    wb[0:84, 6] = np.asarray(Lb2, np.float32).reshape(84)
    wb[0:84, 7:17] = np.asarray(L3, np.float32)
    wb[84, 7:17] = np.asarray(Lb3, np.float32).reshape(10)

    # FC pack [120, 564] (fp32): cols 0:480: FC1 permuted (80 rows);
    # cols 480:564: FC2 (120 rows).
    wfc = np.zeros((120, 564), np.float32)
    for qi in range(4):
        for qj in range(4):
            for oc in range(20):
                wfc[qj * 20 + oc, qi * 120 : (qi + 1) * 120] = L1a[
                    oc * 16 + qi * 4 + qj
                ]
    wfc[:, 480:564] = L2a  # [120, 84]

    return {
        "w8": np.ascontiguousarray(w8, dtype=ml_dtypes.float8_e4m3),
        "wb": np.ascontiguousarray(wb),
        "wfc": np.ascontiguousarray(wfc),
    }


def _build_nc():
    nc = bacc.Bacc()
    xp = nc.dram_tensor("xp", [120, 7, NB], FP8, kind="ExternalInput")
    w8 = nc.dram_tensor("w8", [120, 1088], FP8, kind="ExternalInput")
    wb = nc.dram_tensor("wb", [120, 20], F32R, kind="ExternalInput")
    wfc = nc.dram_tensor("wfc", [120, 564], F32R, kind="ExternalInput")
    y = nc.dram_tensor("y", [10, NB], F32, kind="ExternalOutput")

    with SlimTailTileContext(nc) as tc:
        with (
            tc.tile_pool(name="w", bufs=1) as wp,
            tc.tile_pool(name="act", bufs=1) as ap,
            tc.tile_pool(name="ps", bufs=1, space="PSUM") as psp,
        ):
            # --- SBUF residents ---
            warm = wp.tile([128, 128 + WARM_COLS], BF16, tag="warm")
            w8s = wp.tile([120, 1088], FP8, tag="w8")
            wbs = wp.tile([120, 20], F32R, tag="wb")
            wfcs = wp.tile([120, 564], F32R, tag="wfc")
            xs = ap.tile([120, 7, NB], FP8, tag="xp")
            h1 = ap.tile([120, 12, NB], FP8, tag="h1")
            h2 = ap.tile([80, 4 * NB], F32R, tag="h2")
            h3 = ap.tile([120, NB], F32R, tag="h3")
            h4 = ap.tile([85, NB], F32R, tag="h4")  # row 84 == 1.0 (FC3 bias)
            ys = ap.tile([10, NB], F32, tag="ys")

            t1v = w8s[:, 0:512].rearrange("p (g k) -> p g k", k=128)
            t2v = w8s[:, 512:1088].rearrange("p (g k) -> p g k", k=96)
            b1c = wbs[:, 0:1].bitcast(F32)
            b2col = wbs[0:80, 1:2].bitcast(F32)
            lb1c = wbs[:, 5:6].bitcast(F32)
            lb2c = wbs[0:84, 6:7].bitcast(F32)
            l3s = wbs[0:85, 7:17]
            l1s = wfcs[0:80, 0:480]
            l2s = wfcs[:, 480:564]

            # --- head: Sync HWDGE issues in strict first-use order; the
            # late-needed FC matrices ride GpSimd's SWDGE in parallel.
            # Scalar does no DMA so its ACT table loads run immediately. ---
            nc.gpsimd.memset(warm[:, :], 0.0)
            nc.sync.dma_start(wbs[:], wb[:])
            nc.sync.dma_start(w8s[:], w8[:])
            nc.sync.dma_start(xs[:, 0:2, 0:HB], xp[:, 0:2, 0:HB])
            nc.sync.dma_start(xs[:, 0:2, HB:NB], xp[:, 0:2, HB:NB])
            nc.sync.dma_start(xs[:, 2:4, :], xp[:, 2:4, :])
            nc.sync.dma_start(xs[:, 4:6, :], xp[:, 4:6, :])
            nc.sync.dma_start(xs[:, 6:7, :], xp[:, 6:7, :])
            nc.gpsimd.dma_start(wfcs[:], wfc[:])
            nc.gpsimd.memset(h4[:, :].bitcast(F32), 1.0)
            for _ in range(N_WARM):
                wps = psp.tile([128, WARM_COLS], F32, tag="l1", bufs=4, name="wps")
                nc.tensor.matmul(
                    wps[:], warm[:, :128], warm[:, 128:], start=True, stop=True
                )

            # --- layer 1, per (pi, half): one DoubleRow K=240 matmul into a
            # [128,512] PSUM tile; sigmoid on ACT (even pi, true sigma) or
            # DVE (odd pi, sigma-0.5 via SIG5).  The 4-slot ring maps slots
            # {0,1} to even pi and {2,3} to odd pi, so each slot's WAR chain
            # stays on one sigmoid engine. ---
            def l1_mm(pi, h):
                ps = psp.tile(
                    [128, HB], F32, tag="l1", bufs=4, name=f"psp{pi}_{h}"
                )
                g, p = pi // 2, pi % 2
                b0 = h * HB
                nc.tensor.matmul(
                    ps[:, :],
                    t1v[:, 2 * p : 2 * p + 2, :],
                    xs[:, g : g + 2, b0 : b0 + HB],
                    start=True,
                    stop=True,
                    perf_mode=DR,
                )
                return ps

            def l1_sig(pi, h, ps):
                dst = h1[:, pi, h * HB : h * HB + HB]
                if pi in DVE_PI:
                    nc.vector._custom_dve(
                        SIG5, out=dst, in0=ps[0:120, :], in1=b1c,
                        s0=SC1, s1=SC3, imm2=SC5,
                    )
                else:
                    nc.scalar.activation(dst, ps[0:120, :], SIG, bias=b1c)

            # --- layer 2, per (qi, half): 3 accumulating DoubleRow matmuls
            # into a [96,512] tile from the 4-slot q ring (slots alternate
            # DVE/DVE/ACT/ACT across qi so WAR chains stay on-engine). ---
            l2_ps = {}

            def l2_mms(qi, h, ks):
                key = (qi, h)
                if key not in l2_ps:
                    l2_ps[key] = psp.tile(
                        [96, HB], F32, tag="q", bufs=4, name=f"psq{qi}_{h}"
                    )
                ps = l2_ps[key]
                b0 = h * HB
                for k in ks:
                    nc.tensor.matmul(
                        ps[:, :],
                        t2v[:, 2 * k : 2 * k + 2, :],
                        h1[:, 2 * (qi + k) : 2 * (qi + k) + 2, b0 : b0 + HB],
                        start=(k == 0),
                        stop=(k == 2),
                        perf_mode=DR,
                    )

            def l2_sig(qi, h):
                ps = l2_ps[(qi, h)]
                dst = h2[:, qi * NB + h * HB : qi * NB + h * HB + HB]
                if qi in DVE_QI:
                    nc.vector._custom_dve(
                        SIG5, out=dst, in0=ps[0:80, :], in1=b2col,
                        s0=SC1, s1=SC3, imm2=SC5,
                    )
                else:
                    nc.scalar.activation(dst, ps[0:80, :], SIG, bias=b2col)

            # --- emission: PE runs p halves as x slices land, q matmuls
            # slotted between; all 24 L1 sigmoids are emitted before any q
            # sigmoid on both engines (late p tiles gate q2/q3 and the FC
            # tail). ---
            def l1_pair(pi):
                for h in range(2):
                    l1_sig(pi, h, l1_mm(pi, h))

            for pi in (0, 1, 2, 3, 4, 5):
                l1_pair(pi)
            for h in range(2):
                l2_mms(0, h, (0, 1))
            for pi in (6, 7):
                l1_pair(pi)
            for h in range(2):
                l2_mms(0, h, (2,))
                l2_mms(1, h, (0, 1))
            for pi in (8, 9):
                l1_pair(pi)
            for h in range(2):
                l2_mms(1, h, (2,))
                l2_mms(2, h, (0, 1))
            for pi in (10, 11):
                l1_pair(pi)
            for h in range(2):
                l2_mms(2, h, (2,))
                l2_mms(3, h, (0, 1, 2))
            for qi in range(4):
                for h in range(2):
                    l2_sig(qi, h)

            # --- FC tail, per batch-half: FC1+FC2 ACT sigmoids, FC3 matmul
            # + DVE copy + per-half output DMA.  h0 chains start after the
            # q*h0 sigmoids only. ---
            ps1 = [
                psp.tile([120, HB], F32, tag="l1", bufs=4, name=f"ps1{h}")
                for h in range(2)
            ]
            ps2 = [
                psp.tile([84, HB], F32, tag="q", bufs=4, name=f"ps2{h}")
                for h in range(2)
            ]
            ps3 = [
                psp.tile([10, HB], F32, tag="l1", bufs=4, name=f"ps3{h}")
                for h in range(2)
            ]
            for h in range(2):
                b0 = h * HB
                for qi in range(4):
                    nc.tensor.matmul(
                        ps1[h][:, :],
                        l1s[:, qi * 120 : (qi + 1) * 120],
                        h2[:, qi * NB + b0 : qi * NB + b0 + HB],
                        start=(qi == 0),
                        stop=(qi == 3),
                    )
                nc.scalar.activation(h3[:, b0 : b0 + HB], ps1[h][:, :], SIG, bias=lb1c)
            for h in range(2):
                b0 = h * HB
                nc.tensor.matmul(
                    ps2[h][:, :], l2s, h3[:, b0 : b0 + HB], start=True, stop=True
                )
                nc.scalar.activation(
                    h4[0:84, b0 : b0 + HB], ps2[h][:, :], SIG, bias=lb2c
                )
            for h in range(2):
                b0 = h * HB
                nc.tensor.matmul(
                    ps3[h][:, :], l3s, h4[:, b0 : b0 + HB], start=True, stop=True
                )
                nc.vector.tensor_copy(ys[:, b0 : b0 + HB], ps3[h][:, :])
                nc.sync.dma_start(y[:, b0 : b0 + HB], ys[:, b0 : b0 + HB])
    nc.compile()
    return nc


_NC_CACHE = None


def _get_nc():
    global _NC_CACHE
    if _NC_CACHE is None:
        _NC_CACHE = _build_nc()
    return _NC_CACHE


def _make_in_maps(x, W1, b1, W2, b2, L1, Lb1, L2, Lb2, L3, Lb3):
    wmap = _host_weights(W1, b1, W2, b2, L1, Lb1, L2, Lb2, L3, Lb3)
    x = np.asarray(x, dtype=np.float32)
    in_maps = []
    for c in range(N_CORES):
        xc = x[c * NB : (c + 1) * NB, 0]  # [NB, 28, 28]
        # rows r = 4g + m; partitions: m in {0,1} -> 0:56, m in {2,3} -> 64:120
        v = xc.reshape(NB, 7, 4, 28).transpose(2, 3, 1, 0).reshape(112, 7, NB)
        xpc = np.zeros((120, 7, NB), dtype=ml_dtypes.float8_e4m3)
        xpc[0:56] = v[0:56]
        xpc[64:120] = v[56:112]
        m = {"xp": xpc}
        m.update(wmap)
        in_maps.append(m)
    return in_maps


def _run(trace=False, **inputs):
    global _NC_CACHE
    nc = _get_nc()
    in_maps = _make_in_maps(**inputs)
    res = run_bass_kernel_spmd(nc, in_maps, list(range(N_CORES)), trace=trace)
    # the slim teardown leaves semaphores dirty; force a fresh NEFF if
    # kernel() is ever called again in this process
    _NC_CACHE = None
    outs = []
    for i in range(N_CORES):
        yc = res.results[i]["y"]  # [10, NB]
        outs.append(yc.T)
    out = np.ascontiguousarray(np.concatenate(outs, axis=0))
    return out, res


def kernel(**inputs):
    out, _ = _run(trace=False, **inputs)
    return out


# revision 21
# speedup vs baseline: 1.2047x; 1.0132x over previous
"""LeNet-style CNN (conv5x5+avgpool2+sigmoid x2, then 3 FC layers) on 8 trn2
NeuronCores, pure data parallel over the batch.

v5 key ideas (on top of v2's fused-conv Toeplitz formulation):
- The Activation engine was the measured bottleneck (saturated ~20us sigmoid
  chain; ACT cost = free-size x 0.83ns regardless of dtype).  Half the
  activations now run on the (otherwise idle) DVE as a single custom-DVE op:
  a degree-5 odd minimax polynomial for sigmoid(z)-0.5 (|z|<=1.75 after conv,
  max err 9e-5, far below the fp8 storage noise).  The -0.5 offset is exact:
  it folds into the next layer's bias host-side (b2 += 0.5*sum W2f over the
  DVE-computed kernel rows; Lb1 += 0.5*colsum of the DVE h2 rows of L1).
  The per-partition bias rides in1 via the C3 -> Latch(Src1) spill.
- Everything is computed per batch-half ([*, 512] PSUM tiles): the L1 ring
  (4 bufs) maps even pi to ACT and odd pi to DVE on fixed slots, so the
  write-after-read chain never crosses engines and neither sigmoid engine
  ever waits on the other; same for the 4-slot q ring.  Halving also lets
  FC1's h0 chain start after the q*h0 sigmoids only.
- DMA order is strict first-use order on the fast Sync HWDGE ring (biases,
  conv weights, then x in 5 slices); the late-needed FC matrices ride the
  slow-but-parallel GpSimd SWDGE; the Scalar engine stays a pure sigmoid
  chain so its two auto-inserted ACT table loads finish before data lands.
- 8 short warm-up matmuls bridge the PE from the preamble to first data so
  the HAM clock ramp is never interrupted (an idle gap demotes the PE to
  1.2 GHz for several microseconds).
"""

import numpy as np
import ml_dtypes
import concourse.bacc as bacc
import concourse.mybir as mybir
import concourse.tile as tile
from concourse.vector_clock import ScopedClock
from concourse.bass_utils import run_bass_kernel_spmd

F32 = mybir.dt.float32
F32R = mybir.dt.float32r
BF16 = mybir.dt.bfloat16
FP8 = mybir.dt.float8e4
SIG = mybir.ActivationFunctionType.Sigmoid
DR = mybir.MatmulPerfMode.DoubleRow

N_CORES = 8
B_FULL = 8192
NB = B_FULL // N_CORES  # 1024 images per core
HB = 512  # batch-half: the PSUM tile moving size
N_WARM = 8
WARM_COLS = 256

# sigmoid(z) - 0.5 ~= z*(SC1 + u*(SC3 + u*SC5)), u = z^2; minimax |z|<=1.75,
# max abs err 8.9e-5 (z1 in [-1.16,1.33], z2 in [-0.86,0.80] empirically).
SC1 = 0.2496287852838572
SC3 = -0.019776159138807183
SC5 = 0.0012903995739342435

# pi rows whose L1 sigmoid runs on the DVE (stored as sigma-0.5), and qi
# blocks whose L2 sigmoid runs on the DVE.  Odd pi <=> odd kernel row e for
# every qi, so the b2 correction is qi-independent.
USE_DVE = True
DVE_PI = (1, 3, 5, 7, 9, 11) if USE_DVE else ()
DVE_QI = (0, 2) if USE_DVE else ()


def _register_sig5():
    """Register the SIG5_ANT custom-DVE op (idempotent).  out =
    z*(s0 + u*(s1 + u*imm2)), z = in0 + in1, u = z^2; in1 is the [P,1]
    per-partition bias column, routed via the C3 -> Latch(Src1) spill
    (read once per partition through the swap flop — a bare streaming
    Src1 with a length-1 in1 underruns and hangs the engine)."""
    import concourse.dve_ops as dve_ops
    from concourse.dve_spec import Spec, Src0, C0, C1, C2, C3, sq
    from concourse.dve_spec import lower as spec_lower
    from concourse.dve_spec import _spill_c3_to_src1
    from concourse.dve_uop import DveOpSpec

    if any(op.name == "SIG5_ANT" for op in dve_ops.OPS):
        return next(op for op in dve_ops.OPS if op.name == "SIG5_ANT")

    _z = Src0 + C3
    _u = sq(_z)
    spec = Spec(
        body=_spill_c3_to_src1(_z * (C0 + _u * (C1 + _u * C2))),
        reference=lambda in0, in1, s0, s1, imm2: (
            (in0.astype(np.float32) + in1)
            * (s0 + (in0.astype(np.float32) + in1) ** 2
               * (s1 + (in0.astype(np.float32) + in1) ** 2 * imm2))
        ),
    )
    row = dve_ops._CUSTOM_DVE_ROW_BASE + len(dve_ops.OPS)
    assert row < 0x20
    shas = {}
    for ver in ("v3", "v4"):
        try:
            compiled = DveOpSpec(
                name="SIG5_ANT",
                opcode=row,
                uops=spec_lower(spec, ver=ver),
                rd1_en=True,
            )
            shas[ver] = compiled.sha(ver)
        except Exception:
            pass
    op = dve_ops.DveOp("SIG5_ANT", spec, subdim=False, uops_sha=shas)
    dve_ops.OPS.append(op)
    dve_ops.CUSTOM_DVE_SPECS["SIG5_ANT"] = spec
    dve_ops._SUB_OPCODE_FOR_NAME["SIG5_ANT"] = row
    return op


SIG5 = _register_sig5()


class SlimTailTileContext(tile.TileContext):
    """Tile's standard teardown emits drain + all-engine barrier + semaphore
    clears + another barrier (~10us on HW). This NEFF executes exactly once
    per load, so the semaphore-reset choreography is dead weight: keep the
    data-completeness drain, do the allocator bookkeeping host-side only."""

    def _drain_and_barrier(self, tick_clock, wait_clock):
        drain_inst = self.nc.sync.drain()
        wait_clock.add_sem_waits(
            drain_inst.ins, ScopedClock({None: tick_clock.global_clock})
        )
        popped = self.nc._tile_sem_poison_stack.pop()
        assert popped is self._sem_poison
        sems = list(self.sems.allocated().values())
        sem_nums = [sm.num for sm in sems]
        self.nc._state.prepend_free_semaphores(sem_nums)
        for poison_set in self.nc._tile_sem_poison_stack:
            poison_set.update(sem_nums)


def _fuse_pool(W):
    """conv(W, stride 1) + 2x2 mean-pool == conv(Wf, stride 2), Wf 6x6."""
    O, C, _, _ = W.shape
    Wf = np.zeros((O, C, 6, 6), np.float32)
    for u in (0, 1):
        for v in (0, 1):
            Wf[:, :, u : u + 5, v : v + 5] += W
    return Wf * 0.25


def _host_weights(W1, b1, W2, b2, L1, Lb1, L2, Lb2, L3, Lb3):
    W1f = _fuse_pool(np.asarray(W1, np.float32))  # [10,1,6,6]
    W2f = _fuse_pool(np.asarray(W2, np.float32))  # [20,10,6,6]

    # Layer 1 Toeplitz: S_k[(m',w), (pj,o)] = W1f[o, 0, 2k+m', w-2pj],
    # merged into 4 zero-padded [120,128] stationaries (partition halves
    # 0-55 / 64-119 are the two kernel-row-pair positions of a 4-row group).
    S = np.zeros((3, 56, 120), np.float32)
    for k in range(3):
        for mp in range(2):
            e = 2 * k + mp
            for pj in range(12):
                for f in range(6):
                    w = 2 * pj + f
                    S[k, mp * 28 + w, pj * 10 : pj * 10 + 10] = W1f[:, 0, e, f]
    t1 = np.zeros((120, 4, 128), np.float32)
    t1[0:56, 0, 0:120] = S[0]
    t1[64:120, 0, 0:120] = S[1]
    t1[0:56, 1, 0:120] = S[2]
    t1[64:120, 2, 0:120] = S[0]
    t1[0:56, 3, 0:120] = S[1]
    t1[64:120, 3, 0:120] = S[2]

    # Layer 2 Toeplitz, fp8 DoubleRow layout: block j2 = 2k+par; free dim
    # padded 80 -> 96 for alignment.
    t2 = np.zeros((120, 6, 96), np.float32)
    for par in range(2):
        for k in range(3):
            e = 2 * k + par
            j2 = 2 * k + par
            for qj in range(4):
                for f in range(6):
                    pj = 2 * qj + f
                    for c in range(10):
                        t2[pj * 10 + c, j2, qj * 20 : qj * 20 + 20] = W2f[:, c, e, f]

    w8 = np.zeros((120, 1088), np.float32)
    w8[:, 0:512] = t1.reshape(120, 512)
    w8[:, 512:1088] = t2.reshape(120, 576)

    b1a = np.asarray(b1, np.float32).reshape(10)
    b2a = np.asarray(b2, np.float32).reshape(20)
    L1a = np.asarray(L1, np.float32)
    L2a = np.asarray(L2, np.float32)

    # h1 rows for pi in DVE_PI hold sigma-0.5: the L2 pre-activation is short
    # by 0.5 * sum of the W2f entries on the DVE kernel rows e (pi = 2qi+e so
    # e odd <=> pi odd, independent of qi).
    b2c = b2a.copy()
    if DVE_PI:
        assert DVE_PI == (1, 3, 5, 7, 9, 11)
        b2c = b2a + 0.5 * W2f[:, :, 1::2, :].sum(axis=(1, 2, 3))

    # h2 blocks qi in DVE_QI hold sigma-0.5: FC1 bias correction is
    # 0.5 * column-sum of the corresponding L1 rows (r = oc*16 + qi*4 + qj).
    rows = [
        oc * 16 + qi * 4 + qj for qi in DVE_QI for qj in range(4) for oc in range(20)
    ]
    lb1c = np.asarray(Lb1, np.float32).reshape(120) + (
        0.5 * L1a[rows, :].sum(axis=0) if rows else 0.0
    )

    # bias pack [120, 20] (fp32): col 0: b1 tiled (120); col 1: b2c (80);
    # col 5: lb1' (120); col 6: lb2 (84); cols 7:17: [L3; Lb3] (85 rows).
    wb = np.zeros((120, 20), np.float32)
    wb[:, 0] = np.tile(b1a, 12)
    wb[0:80, 1] = np.tile(b2c, 4)
    wb[:, 5] = lb1c
    wb[0:84, 6] = np.asarray(Lb2, np.float32).reshape(84)
    wb[0:84, 7:17] = np.asarray(L3, np.float32)
    wb[84, 7:17] = np.asarray(Lb3, np.float32).reshape(10)

    # FC pack [120, 564] (fp32): cols 0:480: FC1 permuted (80 rows);
    # cols 480:564: FC2 (120 rows).
    wfc = np.zeros((120, 564), np.float32)
    for qi in range(4):
        for qj in range(4):
            for oc in range(20):
                wfc[qj * 20 + oc, qi * 120 : (qi + 1) * 120] = L1a[
                    oc * 16 + qi * 4 + qj
                ]
    wfc[:, 480:564] = L2a  # [120, 84]

    return {
        "w8": np.ascontiguousarray(w8, dtype=ml_dtypes.float8_e4m3),
        "wb": np.ascontiguousarray(wb),
        "wfc": np.ascontiguousarray(wfc),
    }


def _build_nc():
    nc = bacc.Bacc()
    xp = nc.dram_tensor("xp", [120, 7, NB], FP8, kind="ExternalInput")
    w8 = nc.dram_tensor("w8", [120, 1088], FP8, kind="ExternalInput")
    wb = nc.dram_tensor("wb", [120, 20], F32R, kind="ExternalInput")
    wfc = nc.dram_tensor("wfc", [120, 564], F32R, kind="ExternalInput")
    y = nc.dram_tensor("y", [10, NB], F32, kind="ExternalOutput")

    with SlimTailTileContext(nc) as tc:
        with (
            tc.tile_pool(name="w", bufs=1) as wp,
            tc.tile_pool(name="act", bufs=1) as ap,
            tc.tile_pool(name="ps", bufs=1, space="PSUM") as psp,
        ):
            # --- SBUF residents ---
            warm = wp.tile([128, 128 + WARM_COLS], BF16, tag="warm")
            warmf = wp.tile([128, 16], F32, tag="warmf")
            w8s = wp.tile([120, 1088], FP8, tag="w8")
            wbs = wp.tile([120, 20], F32R, tag="wb")
            wfcs = wp.tile([120, 564], F32R, tag="wfc")
            xs = ap.tile([120, 7, NB], FP8, tag="xp")
            h1 = ap.tile([120, 12, NB], FP8, tag="h1")
            h2 = ap.tile([80, 4 * NB], F32R, tag="h2")
            h3 = ap.tile([120, NB], F32R, tag="h3")
            h4 = ap.tile([85, NB], F32R, tag="h4")  # row 84 == 1.0 (FC3 bias)
            ys = ap.tile([10, NB], F32, tag="ys")

            t1v = w8s[:, 0:512].rearrange("p (g k) -> p g k", k=128)
            t2v = w8s[:, 512:1088].rearrange("p (g k) -> p g k", k=96)
            b1c = wbs[:, 0:1].bitcast(F32)
            b2col = wbs[0:80, 1:2].bitcast(F32)
            lb1c = wbs[:, 5:6].bitcast(F32)
            lb2c = wbs[0:84, 6:7].bitcast(F32)
            l3s = wbs[0:85, 7:17]
            l1s = wfcs[0:80, 0:480]
            l2s = wfcs[:, 480:564]

            # --- head: Sync HWDGE issues in strict first-use order (w8 for
            # the first LDWEIGHTS, then x01, then the bias pack); the
            # late-needed FC matrices ride GpSimd's SWDGE in parallel.
            # Scalar opens with a DMA-independent dummy sigmoid so the
            # auto-inserted ACT table load (1.3us) runs at t~6.3 instead of
            # behind the first bias DMA wait. ---
            nc.gpsimd.memset(warmf[:, :], 0.0)
            nc.gpsimd.memset(warm[:, :], 0.0)
            nc.scalar.activation(warmf[:, 8:16], warmf[:, 0:8], SIG)
            nc.sync.dma_start(w8s[:], w8[:])
            nc.sync.dma_start(xs[:, 0:2, :], xp[:, 0:2, :])
            nc.sync.dma_start(wbs[:], wb[:])
            nc.sync.dma_start(xs[:, 2:4, :], xp[:, 2:4, :])
            nc.sync.dma_start(xs[:, 4:6, :], xp[:, 4:6, :])
            nc.sync.dma_start(xs[:, 6:7, :], xp[:, 6:7, :])
            nc.gpsimd.dma_start(wfcs[:], wfc[:])
            nc.gpsimd.memset(h4[:, :].bitcast(F32), 1.0)
            for _ in range(N_WARM):
                wps = psp.tile([128, WARM_COLS], F32, tag="l1", bufs=4, name="wps")
                nc.tensor.matmul(
                    wps[:], warm[:, :128], warm[:, 128:], start=True, stop=True
                )

            # --- layer 1, per (pi, half): one DoubleRow K=240 matmul into a
            # [128,512] PSUM tile; sigmoid on ACT (even pi, true sigma) or
            # DVE (odd pi, sigma-0.5 via SIG5).  The 4-slot ring maps slots
            # {0,1} to even pi and {2,3} to odd pi, so each slot's WAR chain
            # stays on one sigmoid engine. ---
            def l1_mm(pi, h):
                ps = psp.tile(
                    [128, HB], F32, tag="l1", bufs=4, name=f"psp{pi}_{h}"
                )
                g, p = pi // 2, pi % 2
                b0 = h * HB
                nc.tensor.matmul(
                    ps[:, :],
                    t1v[:, 2 * p : 2 * p + 2, :],
                    xs[:, g : g + 2, b0 : b0 + HB],
                    start=True,
                    stop=True,
                    perf_mode=DR,
                )
                return ps

            def l1_sig(pi, h, ps):
                dst = h1[:, pi, h * HB : h * HB + HB]
                if pi in DVE_PI:
                    nc.vector._custom_dve(
                        SIG5, out=dst, in0=ps[0:120, :], in1=b1c,
                        s0=SC1, s1=SC3, imm2=SC5,
                    )
                else:
                    nc.scalar.activation(dst, ps[0:120, :], SIG, bias=b1c)

            # --- layer 2, per (qi, half): 3 accumulating DoubleRow matmuls
            # into a [96,512] tile from the 4-slot q ring (slots alternate
            # DVE/DVE/ACT/ACT across qi so WAR chains stay on-engine). ---
            l2_ps = {}

            def l2_mms(qi, h, ks):
                key = (qi, h)
                if key not in l2_ps:
                    l2_ps[key] = psp.tile(
                        [96, HB], F32, tag="q", bufs=4, name=f"psq{qi}_{h}"
                    )
                ps = l2_ps[key]
                b0 = h * HB
                for k in ks:
                    nc.tensor.matmul(
                        ps[:, :],
                        t2v[:, 2 * k : 2 * k + 2, :],
                        h1[:, 2 * (qi + k) : 2 * (qi + k) + 2, b0 : b0 + HB],
                        start=(k == 0),
                        stop=(k == 2),
                        perf_mode=DR,
                    )

            def l2_sig(qi, h):
                ps = l2_ps[(qi, h)]
                dst = h2[:, qi * NB + h * HB : qi * NB + h * HB + HB]
                if qi in DVE_QI:
                    nc.vector._custom_dve(
                        SIG5, out=dst, in0=ps[0:80, :], in1=b2col,
                        s0=SC1, s1=SC3, imm2=SC5,
                    )
                else:
                    nc.scalar.activation(dst, ps[0:80, :], SIG, bias=b2col)

            # --- emission: PE runs p halves as x slices land, q matmuls
            # slotted between; all 24 L1 sigmoids are emitted before any q
            # sigmoid on both engines (late p tiles gate q2/q3 and the FC
            # tail). ---
            def l1_pair(pi):
                for h in range(2):
                    l1_sig(pi, h, l1_mm(pi, h))

            for pi in (0, 1, 2, 3, 4, 5):
                l1_pair(pi)
            for h in range(2):
                l2_mms(0, h, (0, 1))
            for pi in (6, 7):
                l1_pair(pi)
            for h in range(2):
                l2_mms(0, h, (2,))
                l2_mms(1, h, (0, 1))
            for pi in (8, 9):
                l1_pair(pi)
            for h in range(2):
                l2_mms(1, h, (2,))
                l2_mms(2, h, (0, 1))
            for pi in (10, 11):
                l1_pair(pi)
            for h in range(2):
                l2_mms(2, h, (2,))
                l2_mms(3, h, (0, 1, 2))
            for qi in range(4):
                for h in range(2):
                    l2_sig(qi, h)

            # --- FC tail, per batch-half: FC1+FC2 ACT sigmoids, FC3 matmul
            # + DVE copy + per-half output DMA.  h0 chains start after the
            # q*h0 sigmoids only. ---
            ps1 = [
                psp.tile([120, HB], F32, tag="l1", bufs=4, name=f"ps1{h}")
                for h in range(2)
            ]
            ps2 = [
                psp.tile([84, HB], F32, tag="q", bufs=4, name=f"ps2{h}")
                for h in range(2)
            ]
            ps3 = [
                psp.tile([10, HB], F32, tag="l1", bufs=4, name=f"ps3{h}")
                for h in range(2)
            ]
            for h in range(2):
                b0 = h * HB
                for qi in range(4):
                    nc.tensor.matmul(
                        ps1[h][:, :],
                        l1s[:, qi * 120 : (qi + 1) * 120],
                        h2[:, qi * NB + b0 : qi * NB + b0 + HB],
                        start=(qi == 0),
                        stop=(qi == 3),
                    )
                nc.scalar.activation(h3[:, b0 : b0 + HB], ps1[h][:, :], SIG, bias=lb1c)
            for h in range(2):
                b0 = h * HB
                nc.tensor.matmul(
                    ps2[h][:, :], l2s, h3[:, b0 : b0 + HB], start=True, stop=True
                )
                nc.scalar.activation(
                    h4[0:84, b0 : b0 + HB], ps2[h][:, :], SIG, bias=lb2c
                )
            for h in range(2):
                b0 = h * HB
                nc.tensor.matmul(
                    ps3[h][:, :], l3s, h4[:, b0 : b0 + HB], start=True, stop=True
                )
                nc.vector.tensor_copy(ys[:, b0 : b0 + HB], ps3[h][:, :])
                nc.sync.dma_start(y[:, b0 : b0 + HB], ys[:, b0 : b0 + HB])
    nc.compile()
    return nc


_NC_CACHE = None


def _get_nc():
    global _NC_CACHE
    if _NC_CACHE is None:
        _NC_CACHE = _build_nc()
    return _NC_CACHE


def _make_in_maps(x, W1, b1, W2, b2, L1, Lb1, L2, Lb2, L3, Lb3):
    wmap = _host_weights(W1, b1, W2, b2, L1, Lb1, L2, Lb2, L3, Lb3)
    x = np.asarray(x, dtype=np.float32)
    in_maps = []
    for c in range(N_CORES):
        xc = x[c * NB : (c + 1) * NB, 0]  # [NB, 28, 28]
        # rows r = 4g + m; partitions: m in {0,1} -> 0:56, m in {2,3} -> 64:120
        v = xc.reshape(NB, 7, 4, 28).transpose(2, 3, 1, 0).reshape(112, 7, NB)
        xpc = np.zeros((120, 7, NB), dtype=ml_dtypes.float8_e4m3)
        xpc[0:56] = v[0:56]
        xpc[64:120] = v[56:112]
        m = {"xp": xpc}
        m.update(wmap)
        in_maps.append(m)
    return in_maps


def _run(trace=False, **inputs):
    global _NC_CACHE
    nc = _get_nc()
    in_maps = _make_in_maps(**inputs)
    res = run_bass_kernel_spmd(nc, in_maps, list(range(N_CORES)), trace=trace)
    # the slim teardown leaves semaphores dirty; force a fresh NEFF if
    # kernel() is ever called again in this process
    _NC_CACHE = None
    outs = []
    for i in range(N_CORES):
        yc = res.results[i]["y"]  # [10, NB]
        outs.append(yc.T)
    out = np.ascontiguousarray(np.concatenate(outs, axis=0))
    return out, res


def kernel(**inputs):
    out, _ = _run(trace=False, **inputs)
    return out


# revision 42
# speedup vs baseline: 1.2854x; 1.0670x over previous
"""LeNet-style CNN (conv5x5+avgpool2+sigmoid x2, then 3 FC layers) on 8 trn2
NeuronCores, pure data parallel over the batch.

v5 key ideas (on top of v2's fused-conv Toeplitz formulation):
- The Activation engine was the measured bottleneck (saturated ~20us sigmoid
  chain; ACT cost = free-size x 0.83ns regardless of dtype).  Half the
  activations now run on the (otherwise idle) DVE as a single custom-DVE op:
  a degree-5 odd minimax polynomial for sigmoid(z)-0.5 (|z|<=1.75 after conv,
  max err 9e-5, far below the fp8 storage noise).  The -0.5 offset is exact:
  it folds into the next layer's bias host-side (b2 += 0.5*sum W2f over the
  DVE-computed kernel rows; Lb1 += 0.5*colsum of the DVE h2 rows of L1).
  The per-partition bias rides in1 via the C3 -> Latch(Src1) spill.
- Everything is computed per batch-half ([*, 512] PSUM tiles): the L1 ring
  (4 bufs) maps even pi to ACT and odd pi to DVE on fixed slots, so the
  write-after-read chain never crosses engines and neither sigmoid engine
  ever waits on the other; same for the 4-slot q ring.  Halving also lets
  FC1's h0 chain start after the q*h0 sigmoids only.
- DMA order is strict first-use order on the fast Sync HWDGE ring (biases,
  conv weights, then x in 5 slices); the late-needed FC matrices ride the
  slow-but-parallel GpSimd SWDGE; the Scalar engine stays a pure sigmoid
  chain so its two auto-inserted ACT table loads finish before data lands.
- 8 short warm-up matmuls bridge the PE from the preamble to first data so
  the HAM clock ramp is never interrupted (an idle gap demotes the PE to
  1.2 GHz for several microseconds).
"""

import numpy as np
import ml_dtypes
import concourse.bacc as bacc
import concourse.mybir as mybir
import concourse.tile as tile
from concourse.vector_clock import ScopedClock
from concourse.bass_utils import run_bass_kernel_spmd

F32 = mybir.dt.float32
F32R = mybir.dt.float32r
BF16 = mybir.dt.bfloat16
FP8 = mybir.dt.float8e4
SIG = mybir.ActivationFunctionType.Sigmoid
DR = mybir.MatmulPerfMode.DoubleRow

N_CORES = 8
B_FULL = 8192
NB = B_FULL // N_CORES  # 1024 images per core
HB = 512  # batch-half: the PSUM tile moving size
N_WARM = 9
WARM_COLS = 256

# sigmoid(z) - 0.5 ~= z*(SC1 + u*(SC3 + u*SC5)), u = z^2; minimax |z|<=1.75,
# max abs err 8.9e-5 (z1 in [-1.16,1.33], z2 in [-0.86,0.80] empirically).
SC1 = 0.2496287852838572
SC3 = -0.019776159138807183
SC5 = 0.0012903995739342435

# (pi, half) tiles whose L1 sigmoid runs on the DVE (stored as sigma-0.5),
# and qi blocks whose L2 sigmoid runs on the DVE.  Odd pi <=> odd kernel row
# e for every qi, so the odd-pi b2 correction is qi-independent; the extra
# (0, h1) tile on the DVE (engine load balance) only affects qi=0's h1
# correction, which gets its own bias column.
USE_DVE = True
DVE_PI = (1, 3, 5, 7, 9, 11) if USE_DVE else ()
EXTRA_DVE_H1_PI = (0,) if USE_DVE else ()
DVE_QI = (0, 2) if USE_DVE else ()


def _is_dve_l1(pi, h):
    return pi in DVE_PI or (h == 1 and pi in EXTRA_DVE_H1_PI)


def _register_sig5():
    """Register the SIG5_ANT custom-DVE op (idempotent).  out =
    z*(s0 + u*(s1 + u*imm2)), z = in0 + in1, u = z^2; in1 is the [P,1]
    per-partition bias column, routed via the C3 -> Latch(Src1) spill
    (read once per partition through the swap flop — a bare streaming
    Src1 with a length-1 in1 underruns and hangs the engine)."""
    import concourse.dve_ops as dve_ops
    from concourse.dve_spec import Spec, Src0, C0, C1, C2, C3, sq
    from concourse.dve_spec import lower as spec_lower
    from concourse.dve_spec import _spill_c3_to_src1
    from concourse.dve_uop import DveOpSpec

    if any(op.name == "SIG5_ANT" for op in dve_ops.OPS):
        return next(op for op in dve_ops.OPS if op.name == "SIG5_ANT")

    _z = Src0 + C3
    _u = sq(_z)
    spec = Spec(
        body=_spill_c3_to_src1(_z * (C0 + _u * (C1 + _u * C2))),
        reference=lambda in0, in1, s0, s1, imm2: (
            (in0.astype(np.float32) + in1)
            * (s0 + (in0.astype(np.float32) + in1) ** 2
               * (s1 + (in0.astype(np.float32) + in1) ** 2 * imm2))
        ),
    )
    row = dve_ops._CUSTOM_DVE_ROW_BASE + len(dve_ops.OPS)
    assert row < 0x20
    shas = {}
    for ver in ("v3", "v4"):
        try:
            compiled = DveOpSpec(
                name="SIG5_ANT",
                opcode=row,
                uops=spec_lower(spec, ver=ver),
                rd1_en=True,
            )
            shas[ver] = compiled.sha(ver)
        except Exception:
            pass
    op = dve_ops.DveOp("SIG5_ANT", spec, subdim=False, uops_sha=shas)
    dve_ops.OPS.append(op)
    dve_ops.CUSTOM_DVE_SPECS["SIG5_ANT"] = spec
    dve_ops._SUB_OPCODE_FOR_NAME["SIG5_ANT"] = row
    return op


SIG5 = _register_sig5()


class SlimTailTileContext(tile.TileContext):
    """Tile's standard teardown emits drain + all-engine barrier + semaphore
    clears + another barrier (~10us on HW). This NEFF executes exactly once
    per load, so the semaphore-reset choreography is dead weight: keep the
    data-completeness drain, do the allocator bookkeeping host-side only."""

    def _drain_and_barrier(self, tick_clock, wait_clock):
        drain_inst = self.nc.sync.drain()
        wait_clock.add_sem_waits(
            drain_inst.ins, ScopedClock({None: tick_clock.global_clock})
        )
        popped = self.nc._tile_sem_poison_stack.pop()
        assert popped is self._sem_poison
        sems = list(self.sems.allocated().values())
        sem_nums = [sm.num for sm in sems]
        self.nc._state.prepend_free_semaphores(sem_nums)
        for poison_set in self.nc._tile_sem_poison_stack:
            poison_set.update(sem_nums)


def _fuse_pool(W):
    """conv(W, stride 1) + 2x2 mean-pool == conv(Wf, stride 2), Wf 6x6."""
    O, C, _, _ = W.shape
    Wf = np.zeros((O, C, 6, 6), np.float32)
    for u in (0, 1):
        for v in (0, 1):
            Wf[:, :, u : u + 5, v : v + 5] += W
    return Wf * 0.25


def _host_weights(W1, b1, W2, b2, L1, Lb1, L2, Lb2, L3, Lb3):
    W1f = _fuse_pool(np.asarray(W1, np.float32))  # [10,1,6,6]
    W2f = _fuse_pool(np.asarray(W2, np.float32))  # [20,10,6,6]

    # Layer 1 Toeplitz: S_k[(m',w), (pj,o)] = W1f[o, 0, 2k+m', w-2pj],
    # merged into 4 zero-padded [120,128] stationaries (partition halves
    # 0-55 / 64-119 are the two kernel-row-pair positions of a 4-row group).
    S = np.zeros((3, 56, 120), np.float32)
    for k in range(3):
        for mp in range(2):
            e = 2 * k + mp
            for pj in range(12):
                for f in range(6):
                    w = 2 * pj + f
                    S[k, mp * 28 + w, pj * 10 : pj * 10 + 10] = W1f[:, 0, e, f]
    t1 = np.zeros((120, 4, 128), np.float32)
    t1[0:56, 0, 0:120] = S[0]
    t1[64:120, 0, 0:120] = S[1]
    t1[0:56, 1, 0:120] = S[2]
    t1[64:120, 2, 0:120] = S[0]
    t1[0:56, 3, 0:120] = S[1]
    t1[64:120, 3, 0:120] = S[2]

    # Layer 2 Toeplitz, fp8 DoubleRow layout: block j2 = 2k+par; free dim
    # padded 80 -> 96 for alignment.
    t2 = np.zeros((120, 6, 96), np.float32)
    for par in range(2):
        for k in range(3):
            e = 2 * k + par
            j2 = 2 * k + par
            for qj in range(4):
                for f in range(6):
                    pj = 2 * qj + f
                    for c in range(10):
                        t2[pj * 10 + c, j2, qj * 20 : qj * 20 + 20] = W2f[:, c, e, f]

    w8 = np.zeros((120, 1088), np.float32)
    w8[:, 0:512] = t1.reshape(120, 512)
    w8[:, 512:1088] = t2.reshape(120, 576)

    b1a = np.asarray(b1, np.float32).reshape(10)
    b2a = np.asarray(b2, np.float32).reshape(20)
    L1a = np.asarray(L1, np.float32)
    L2a = np.asarray(L2, np.float32)

    # h1 rows for DVE (pi, h) tiles hold sigma-0.5: each qi's L2
    # pre-activation is short by 0.5 * sum of the W2f entries on the DVE
    # kernel rows e (pi = 2qi+e, so e odd <=> pi odd for every qi, plus the
    # balance tile (pi=0, h1) which is row e=0 of qi=0 only).
    b2c_h0 = b2a.copy()
    b2c_h1q0 = b2a.copy()
    b2c_h1 = b2a.copy()
    if DVE_PI:
        assert DVE_PI == (1, 3, 5, 7, 9, 11)
        odd = 0.5 * W2f[:, :, 1::2, :].sum(axis=(1, 2, 3))
        b2c_h0 = b2a + odd
        b2c_h1 = b2a + odd
        b2c_h1q0 = b2a + odd
        if EXTRA_DVE_H1_PI:
            assert EXTRA_DVE_H1_PI == (0,)
            b2c_h1q0 = b2c_h1q0 + 0.5 * W2f[:, :, 0, :].sum(axis=(1, 2))

    # h2 blocks qi in DVE_QI hold sigma-0.5: FC1 bias correction is
    # 0.5 * column-sum of the corresponding L1 rows (r = oc*16 + qi*4 + qj).
    rows = [
        oc * 16 + qi * 4 + qj for qi in DVE_QI for qj in range(4) for oc in range(20)
    ]
    lb1c = np.asarray(Lb1, np.float32).reshape(120) + (
        0.5 * L1a[rows, :].sum(axis=0) if rows else 0.0
    )

    # bias pack [120, 20] (fp32): col 0: b1 tiled (120); cols 1-3: b2c for
    # (h0), (h1, qi=0), (h1, qi>0) (80 each); col 5: lb1' (120); col 6: lb2
    # (84); cols 7:17: [L3; Lb3] (85 rows).
    wb = np.zeros((120, 20), np.float32)
    wb[:, 0] = np.tile(b1a, 12)
    wb[0:80, 1] = np.tile(b2c_h0, 4)
    wb[0:80, 2] = np.tile(b2c_h1q0, 4)
    wb[0:80, 3] = np.tile(b2c_h1, 4)
    wb[:, 5] = lb1c
    wb[0:84, 6] = np.asarray(Lb2, np.float32).reshape(84)
    wb[0:84, 7:17] = np.asarray(L3, np.float32)
    wb[84, 7:17] = np.asarray(Lb3, np.float32).reshape(10)

    # FC pack [120, 564] (fp32): cols 0:480: FC1 permuted (80 rows);
    # cols 480:564: FC2 (120 rows).
    wfc = np.zeros((120, 564), np.float32)
    for qi in range(4):
        for qj in range(4):
            for oc in range(20):
                wfc[qj * 20 + oc, qi * 120 : (qi + 1) * 120] = L1a[
                    oc * 16 + qi * 4 + qj
                ]
    wfc[:, 480:564] = L2a  # [120, 84]

    return {
        "w8": np.ascontiguousarray(w8, dtype=ml_dtypes.float8_e4m3),
        "wb": np.ascontiguousarray(wb),
        "wfc": np.ascontiguousarray(wfc),
    }


def _build_nc():
    nc = bacc.Bacc()
    xp = nc.dram_tensor("xp", [120, 7, NB], FP8, kind="ExternalInput")
    w8 = nc.dram_tensor("w8", [120, 1088], FP8, kind="ExternalInput")
    wb = nc.dram_tensor("wb", [120, 20], F32R, kind="ExternalInput")
    wfc = nc.dram_tensor("wfc", [120, 564], F32R, kind="ExternalInput")
    y = nc.dram_tensor("y", [10, NB], F32, kind="ExternalOutput")

    with SlimTailTileContext(nc) as tc:
        with (
            tc.tile_pool(name="w", bufs=1) as wp,
            tc.tile_pool(name="act", bufs=1) as ap,
            tc.tile_pool(name="ps", bufs=1, space="PSUM") as psp,
        ):
            # --- SBUF residents ---
            warm = wp.tile([128, 128 + WARM_COLS], BF16, tag="warm")
            warmf = wp.tile([128, 16], F32, tag="warmf")
            w8s = wp.tile([120, 1088], FP8, tag="w8")
            wbs = wp.tile([120, 20], F32R, tag="wb")
            wfcs = wp.tile([120, 564], F32R, tag="wfc")
            xs = ap.tile([120, 7, NB], FP8, tag="xp")
            h1 = ap.tile([120, 12, NB], FP8, tag="h1")
            h2 = ap.tile([80, 4 * NB], F32R, tag="h2")
            h3 = ap.tile([120, NB], F32R, tag="h3")
            h4 = ap.tile([85, NB], F32R, tag="h4")  # row 84 == 1.0 (FC3 bias)
            ys = ap.tile([10, NB], F32, tag="ys")

            t1v = w8s[:, 0:512].rearrange("p (g k) -> p g k", k=128)
            t2v = w8s[:, 512:1088].rearrange("p (g k) -> p g k", k=96)
            b1c = wbs[:, 0:1].bitcast(F32)

            def b2col(qi, h):
                col = 1 if h == 0 else (2 if qi == 0 else 3)
                return wbs[0:80, col : col + 1].bitcast(F32)

            lb1c = wbs[:, 5:6].bitcast(F32)
            lb2c = wbs[0:84, 6:7].bitcast(F32)
            l3s = wbs[0:85, 7:17]
            l1s = wfcs[0:80, 0:480]
            l2s = wfcs[:, 480:564]

            # --- head: Sync HWDGE issues in strict first-use order (w8 for
            # the first LDWEIGHTS, then x01, then the bias pack); the
            # late-needed FC matrices ride GpSimd's SWDGE in parallel.
            # Scalar opens with a DMA-independent dummy sigmoid so the
            # auto-inserted ACT table load (1.3us) runs at t~6.3 instead of
            # behind the first bias DMA wait. ---
            nc.gpsimd.memset(warmf[:, :], 0.0)
            nc.gpsimd.memset(warm[:, :], 0.0)
            nc.scalar.dma_start(w8s[:], w8[:])
            nc.scalar.activation(warmf[:, 8:16], warmf[:, 0:8], SIG)
            nc.sync.dma_start(xs[:, 0:2, :], xp[:, 0:2, :])
            nc.sync.dma_start(wbs[:], wb[:])
            nc.sync.dma_start(xs[:, 2:4, :], xp[:, 2:4, :])
            nc.sync.dma_start(xs[:, 4:6, :], xp[:, 4:6, :])
            nc.sync.dma_start(xs[:, 6:7, :], xp[:, 6:7, :])
            nc.sync.dma_start(wfcs[:], wfc[:])
            nc.gpsimd.memset(h4[:, :].bitcast(F32), 1.0)
            for _ in range(N_WARM):
                wps = psp.tile([128, WARM_COLS], F32, tag="l1", bufs=4, name="wps")
                nc.tensor.matmul(
                    wps[:], warm[:, :128], warm[:, 128:], start=True, stop=True
                )

            # --- layer 1, per (pi, half): one DoubleRow K=240 matmul into a
            # [128,512] PSUM tile; sigmoid on ACT (even pi, true sigma) or
            # DVE (odd pi, sigma-0.5 via SIG5).  The 4-slot ring maps slots
            # {0,1} to even pi and {2,3} to odd pi, so each slot's WAR chain
            # stays on one sigmoid engine. ---
            def l1_mm(pi, h):
                ps = psp.tile(
                    [128, HB], F32, tag="l1", bufs=4, name=f"psp{pi}_{h}"
                )
                g, p = pi // 2, pi % 2
                b0 = h * HB
                nc.tensor.matmul(
                    ps[:, :],
                    t1v[:, 2 * p : 2 * p + 2, :],
                    xs[:, g : g + 2, b0 : b0 + HB],
                    start=True,
                    stop=True,
                    perf_mode=DR,
                )
                return ps

            def l1_sig(pi, h, ps):
                dst = h1[:, pi, h * HB : h * HB + HB]
                if _is_dve_l1(pi, h):
                    nc.vector._custom_dve(
                        SIG5, out=dst, in0=ps[0:120, :], in1=b1c,
                        s0=SC1, s1=SC3, imm2=SC5,
                    )
                else:
                    nc.scalar.activation(dst, ps[0:120, :], SIG, bias=b1c)

            # --- layer 2, per (qi, half): 3 accumulating DoubleRow matmuls
            # into a [96,512] tile from the 4-slot q ring (slots alternate
            # DVE/DVE/ACT/ACT across qi so WAR chains stay on-engine). ---
            l2_ps = {}

            def l2_mms(qi, h, ks):
                key = (qi, h)
                if key not in l2_ps:
                    l2_ps[key] = psp.tile(
                        [96, HB], F32, tag="q", bufs=4, name=f"psq{qi}_{h}"
                    )
                ps = l2_ps[key]
                b0 = h * HB
                for k in ks:
                    nc.tensor.matmul(
                        ps[:, :],
                        t2v[:, 2 * k : 2 * k + 2, :],
                        h1[:, 2 * (qi + k) : 2 * (qi + k) + 2, b0 : b0 + HB],
                        start=(k == 0),
                        stop=(k == 2),
                        perf_mode=DR,
                    )

            def l2_sig(qi, h):
                ps = l2_ps[(qi, h)]
                dst = h2[:, qi * NB + h * HB : qi * NB + h * HB + HB]
                if qi in DVE_QI:
                    nc.vector._custom_dve(
                        SIG5, out=dst, in0=ps[0:80, :], in1=b2col(qi, h),
                        s0=SC1, s1=SC3, imm2=SC5,
                    )
                else:
                    nc.scalar.activation(dst, ps[0:80, :], SIG, bias=b2col(qi, h))

            # --- emission: PE runs p halves as x slices land, q matmuls
            # slotted between; all 24 L1 sigmoids are emitted before any q
            # sigmoid on both engines (late p tiles gate q2/q3 and the FC
            # tail). ---
            def l1_pair(pi):
                for h in range(2):
                    l1_sig(pi, h, l1_mm(pi, h))

            for pi in (0, 1, 2, 3, 4, 5):
                l1_pair(pi)
            for h in range(2):
                l2_mms(0, h, (0, 1))
            for pi in (6, 7):
                l1_pair(pi)
            for h in range(2):
                l2_mms(0, h, (2,))
                l2_mms(1, h, (0, 1))
            for pi in (8, 9):
                l1_pair(pi)
            for h in range(2):
                l2_mms(1, h, (2,))
                l2_mms(2, h, (0, 1))
            for pi in (10, 11):
                l1_pair(pi)
            for h in range(2):
                l2_mms(2, h, (2,))
                l2_mms(3, h, (0, 1, 2))
            # h0 q-blocks first on both engines: FC1's h0 chain only needs
            # the h0 sigmoids, so it overlaps the h1 q round.
            for h in range(2):
                for qi in range(4):
                    l2_sig(qi, h)

            # --- FC tail, per batch-half: FC1+FC2 ACT sigmoids, FC3 matmul
            # + DVE copy + per-half output DMA.  h0 chains start after the
            # q*h0 sigmoids only. ---
            ps1 = [
                psp.tile([120, HB], F32, tag="l1", bufs=4, name=f"ps1{h}")
                for h in range(2)
            ]
            ps2 = [
                psp.tile([84, HB], F32, tag="q", bufs=4, name=f"ps2{h}")
                for h in range(2)
            ]
            ps3 = [
                psp.tile([10, HB], F32, tag="l1", bufs=4, name=f"ps3{h}")
                for h in range(2)
            ]
            for h in range(2):
                b0 = h * HB
                for qi in range(4):
                    nc.tensor.matmul(
                        ps1[h][:, :],
                        l1s[:, qi * 120 : (qi + 1) * 120],
                        h2[:, qi * NB + b0 : qi * NB + b0 + HB],
                        start=(qi == 0),
                        stop=(qi == 3),
                    )
                nc.scalar.activation(h3[:, b0 : b0 + HB], ps1[h][:, :], SIG, bias=lb1c)
            for h in range(2):
                b0 = h * HB
                nc.tensor.matmul(
                    ps2[h][:, :], l2s, h3[:, b0 : b0 + HB], start=True, stop=True
                )
                nc.scalar.activation(
                    h4[0:84, b0 : b0 + HB], ps2[h][:, :], SIG, bias=lb2c
                )
            # FC3: the two halves' PSUM->SBUF copies run on different engines
            # (DVE h0, GpSimd h1) and the output DMAs issue from different
            # rings (Sync h0, Scalar h1) so the last-half path is short.
            for h in range(2):
                b0 = h * HB
                nc.tensor.matmul(
                    ps3[h][:, :], l3s, h4[:, b0 : b0 + HB], start=True, stop=True
                )
            nc.vector.tensor_copy(ys[:, 0:HB], ps3[0][:, :])
            nc.scalar.activation(
                ys[:, HB:NB], ps3[1][:, :], mybir.ActivationFunctionType.Copy
            )
            nc.sync.dma_start(y[:, 0:HB], ys[:, 0:HB])
            nc.scalar.dma_start(y[:, HB:NB], ys[:, HB:NB])
    nc.compile()
    return nc


_NC_CACHE = None


def _get_nc():
    global _NC_CACHE
    if _NC_CACHE is None:
        _NC_CACHE = _build_nc()
    return _NC_CACHE


def _make_in_maps(x, W1, b1, W2, b2, L1, Lb1, L2, Lb2, L3, Lb3):
    wmap = _host_weights(W1, b1, W2, b2, L1, Lb1, L2, Lb2, L3, Lb3)
    x = np.asarray(x, dtype=np.float32)
    in_maps = []
    for c in range(N_CORES):
        xc = x[c * NB : (c + 1) * NB, 0]  # [NB, 28, 28]
        # rows r = 4g + m; partitions: m in {0,1} -> 0:56, m in {2,3} -> 64:120
        v = xc.reshape(NB, 7, 4, 28).transpose(2, 3, 1, 0).reshape(112, 7, NB)
        xpc = np.zeros((120, 7, NB), dtype=ml_dtypes.float8_e4m3)
        xpc[0:56] = v[0:56]
        xpc[64:120] = v[56:112]
        m = {"xp": xpc}
        m.update(wmap)
        in_maps.append(m)
    return in_maps


def _run(trace=False, **inputs):
    global _NC_CACHE
    nc = _get_nc()
    in_maps = _make_in_maps(**inputs)
    res = run_bass_kernel_spmd(nc, in_maps, list(range(N_CORES)), trace=trace)
    # the slim teardown leaves semaphores dirty; force a fresh NEFF if
    # kernel() is ever called again in this process
    _NC_CACHE = None
    outs = []
    for i in range(N_CORES):
        yc = res.results[i]["y"]  # [10, NB]
        outs.append(yc.T)
    out = np.ascontiguousarray(np.concatenate(outs, axis=0))
    return out, res


def kernel(**inputs):
    out, _ = _run(trace=False, **inputs)
    return out
